# revision 1
# baseline (speedup 1.0000x reference)
"""nn_Attn_9715216024104 — sparse attention (MLA + top-k select + sliding window).

Sharding: 8 cores = 2 batches x 4 head-groups (4 heads each). Each core runs
one Bass/Tile kernel computing its 4 heads' three attention branches
(S^T layout, exp softmax without max-subtraction — scores are <0.5 — with
ones-column-folded Z rows in the PV matmul). The device returns raw per-
(branch, head) OT+Z tiles; the host epilogue normalizes by Z, applies the
branch gate, projects through Wproj, and sums head-group partials. Host also
does the tiny shared prep (cq/ckv RMS, shared-kr rope, top-k select, gate
softmax, transposes, bf16 casts).

Device layout notes:
- All matmul operands bf16; PSUM accumulation f32.
- Attention uses S^T tiles [k=128, q] so P^T feeds the PV matmul directly;
  V tiles carry a ones column so the PV matmul also produces Z rows.
- Rope is applied via duplicated "swapped" projection weights:
  rope(x) = x * cos + swap(x) * sgn*sin, with swap folded into a second
  matmul (host reorders weight columns), so DVE only does 2 muls + 1 add.
- Causal / sliding-window masking is done on GPSIMD (affine_select zeroing
  of P^T after exp), keeping TensorE/ACT free of mask work.
"""

import math

import numpy as np
import ml_dtypes

BF = ml_dtypes.bfloat16

N_HEAD = 16
NOPE = 32
ROPE = 64
VDIM = 32
HD = NOPE + ROPE  # 96
WINDOW = 128
KEEP = 512
EPS = 1e-6
N_CORES = 8
HPC = 4  # heads per core
B, T, C = 2, 2048, 256
QT = 512  # q tile (free dim)
NJQ = T // QT  # 4 q tiles
NKB = T // 128  # 16 k blocks
SCALE = 1.0 / math.sqrt(HD)

_CACHE = {}


# ---------------------------------------------------------------------------
# host-side helpers
# ---------------------------------------------------------------------------

def _freqs(t):
    f = 1.0 / 1e4 ** (np.arange(0, ROPE, 2, dtype=np.float32) / ROPE)
    ang = np.outer(np.arange(t, dtype=np.float32), f)
    return np.cos(ang).astype(np.float32), np.sin(ang).astype(np.float32)


def _rms(x, w):
    return x * (1.0 / np.sqrt(np.mean(x * x, -1, keepdims=True) + EPS)) * w


def _rope_host(x, cos, sin):
    t = x.shape[0]
    xr, xi = x[:, : x.shape[1] // 2], x[:, x.shape[1] // 2 :]
    return np.concatenate(
        [xr * cos[:t] - xi * sin[:t], xr * sin[:t] + xi * cos[:t]], -1
    )


def _rope_tables():
    cos, sin = _freqs(T)  # [T, 32]
    c128 = np.tile(cos.T, (4, 1))  # [128, T]
    sgn = np.repeat(np.array([-1.0, 1.0], np.float32), 32)
    sgn = np.tile(sgn, 2)[:, None]  # [-1]*32, [+1]*32, [-1]*32, [+1]*32
    s128 = np.tile(sin.T, (4, 1)) * sgn
    return c128.astype(BF), s128.astype(BF)


def _swap_cols(w):
    """Swap (real, imag) halves of each head's rope columns. w [cin, nh, ROPE]."""
    return np.concatenate([w[..., ROPE // 2 :], w[..., : ROPE // 2]], -1)


# ---------------------------------------------------------------------------
# bass program (built once; identical for all 8 cores)
# ---------------------------------------------------------------------------

def _build_bass(legalize=True):
    import concourse.bass as bass
    import concourse.mybir as mybir
    import concourse.tile as tile

    f32 = mybir.dt.float32
    bf16 = mybir.dt.bfloat16
    EXP = mybir.ActivationFunctionType.Exp
    GE = mybir.AluOpType.is_ge

    nc = bass.Bass(target_bir_lowering=False, debug=False)

    def inp(name, shape):
        return nc.declare_dram_parameter(name, list(shape), bf16, isOutput=False)

    d_xT = inp("xT", (2, 128, T))
    d_cqT = inp("cqT", (96, T))
    d_ckvT = inp("ckvT", (32, T))
    d_krT = inp("krT", (64, T))
    d_selT = inp("selT", (2, 128, KEEP))
    d_Wqn = inp("Wqn", (96, 128))
    d_Wqr = inp("Wqr", (96, 256))
    d_WqrS = inp("WqrS", (96, 256))
    d_Wkn = inp("Wkn", (32, 128))
    d_Wv = inp("Wv", (32, 128))
    d_Wskn = inp("Wskn", (2, 128, 128))
    d_Wskr = inp("Wskr", (2, 128, 256))
    d_WskrS = inp("WskrS", (2, 128, 256))
    d_Wsv = inp("Wsv", (2, 128, 128))
    d_Wwkn = inp("Wwkn", (2, 128, 128))
    d_Wwkr = inp("Wwkr", (2, 128, 256))
    d_WwkrS = inp("WwkrS", (2, 128, 256))
    d_Wwv = inp("Wwv", (2, 128, 128))
    d_cos = inp("cosT", (128, T))
    d_sin = inp("sinT", (128, T))  # sign-folded
    d_out = nc.declare_dram_parameter("outT", [NJQ, 12, 33, QT],
                                      bf16, isOutput=True)

    with tile.TileContext(nc) as tc:
        with (
            tc.tile_pool(name="const", bufs=1) as cpool,
            tc.tile_pool(name="big", bufs=1) as bpool,
            tc.tile_pool(name="pt", bufs=3) as ptpool,
            tc.tile_pool(name="sc", bufs=4) as scpool,
        ):
            # ---- load inputs to SBUF (spread across engine DMA queues) ----
            _dma_engines = [nc.sync, nc.gpsimd, nc.scalar]
            _dma_rr = [0]

            def _dma(out, in_):
                eng = _dma_engines[_dma_rr[0] % len(_dma_engines)]
                _dma_rr[0] += 1
                eng.dma_start(out=out, in_=in_)

            def load2(name, dram, shape):
                """[128, 2, X] sbuf tile from [2, 128, X] dram."""
                t = cpool.tile([128, 2, shape[2]], bf16, name=name, tag=name)
                for cc in range(2):
                    _dma(t[:, cc, :], dram[cc])
                return t

            def load1(name, dram, shape):
                t = cpool.tile(list(shape), bf16, name=name, tag=name)
                _dma(t[:], dram[:])
                return t

            s_cqT = load1("cqT", d_cqT, (96, T))
            s_Wqn = load1("Wqn", d_Wqn, (96, 128))
            s_Wqr = load1("Wqr", d_Wqr, (96, 256))
            s_WqrS = load1("WqrS", d_WqrS, (96, 256))
            s_ckvT = load1("ckvT", d_ckvT, (32, T))
            s_Wkn = load1("Wkn", d_Wkn, (32, 128))
            s_Wv = load1("Wv", d_Wv, (32, 128))
            s_cos = load1("cosT", d_cos, (128, T))
            s_sin = load1("sinT", d_sin, (128, T))
            s_selT = load2("selT", d_selT, (2, 128, KEEP))
            s_Wskn = load2("Wskn", d_Wskn, (2, 128, 128))
            s_Wskr = load2("Wskr", d_Wskr, (2, 128, 256))
            s_WskrS = load2("WskrS", d_WskrS, (2, 128, 256))
            s_Wsv = load2("Wsv", d_Wsv, (2, 128, 128))
            s_xT = load2("xT", d_xT, (2, 128, T))
            s_Wwkn = load2("Wwkn", d_Wwkn, (2, 128, 128))
            s_Wwkr = load2("Wwkr", d_Wwkr, (2, 128, 256))
            s_WwkrS = load2("WwkrS", d_WwkrS, (2, 128, 256))
            s_Wwv = load2("Wwv", d_Wwv, (2, 128, 128))

            # ---- assembled per-head [96, h, T] q/k layouts ----
            q96 = bpool.tile([96, 4, T], bf16)
            k96 = bpool.tile([96, 4, T], bf16)     # branch 1 (kn | shared kr)
            ks96 = bpool.tile([96, 4, KEEP], bf16)  # branch 2
            kw96 = bpool.tile([96, 4, T], bf16)    # branch 3
            v1 = bpool.tile([128, NKB, 132], bf16)
            vs = bpool.tile([128, 4, 132], bf16)
            vw = bpool.tile([128, NKB, 132], bf16)

            # shared roped kr rows broadcast into all 4 heads of k96
            for h in range(4):
                _dma(k96[0:64, h, :], d_krT[:])

            with (
                tc.tile_pool(name="pp", bufs=2, space=bass.MemorySpace.PSUM) as pp,
                tc.tile_pool(name="sgp", bufs=2, space=bass.MemorySpace.PSUM) as sgp,
                tc.tile_pool(name="otp", bufs=2, space=bass.MemorySpace.PSUM) as otp,
            ):
                def proj_nope(dest96, lhsW, rhs_of, tlen, nacc, eng=None):
                    """4-head nope projection, split per head into
                    dest96[64:96, h, ts]."""
                    step = min(tlen, QT)
                    for t4 in range(max(1, tlen // step)):
                        ts_ = slice(t4 * step, t4 * step + step)
                        ps = pp.tile([128, step], f32, tag="p1",
                                     padded_shape=[128, QT])
                        for cc in range(nacc):
                            nc.tensor.matmul(
                                ps[:], lhsW(cc), rhs_of(cc, ts_),
                                start=(cc == 0), stop=(cc == nacc - 1),
                            )
                        for h in range(4):
                            if eng == "act":
                                nc.scalar.copy(
                                    dest96[64:96, h, ts_],
                                    ps[32 * h : 32 * h + 32, :],
                                )
                            else:
                                nc.vector.tensor_copy(
                                    dest96[64:96, h, ts_],
                                    ps[32 * h : 32 * h + 32, :],
                                )

                def rope_proj(dest96, hpair, cos_sl, lhs_raw, lhs_sw,
                              rhs_list, tlen, ts_):
                    """Rope for one head-pair chunk; writes per-head rows
                    dest96[32:96, h, ts]."""
                    pr = pp.tile([128, tlen], f32, tag="p1",
                                 padded_shape=[128, QT])
                    psw = pp.tile([128, tlen], f32, tag="p1",
                                  padded_shape=[128, QT])
                    ncc = len(rhs_list)
                    for cc, rhs in enumerate(rhs_list):
                        nc.tensor.matmul(
                            pr[:], lhs_raw[cc], rhs,
                            start=(cc == 0), stop=(cc == ncc - 1),
                        )
                    for cc, rhs in enumerate(rhs_list):
                        nc.tensor.matmul(
                            psw[:], lhs_sw[cc], rhs,
                            start=(cc == 0), stop=(cc == ncc - 1),
                        )
                    t1 = scpool.tile([128, tlen], bf16, tag="rt1")
                    t2 = scpool.tile([128, tlen], bf16, tag="rt2")
                    nc.vector.tensor_mul(t1[:], pr[:], s_cos[:, cos_sl])
                    nc.vector.tensor_mul(t2[:], psw[:], s_sin[:, cos_sl])
                    for hi in range(2):
                        h = 2 * hpair + hi
                        hr = slice(64 * hi, 64 * hi + 64)
                        nc.gpsimd.tensor_add(
                            dest96[0:64, h, ts_], t1[hr, :], t2[hr, :]
                        )

                def rope_all(dest96, lhsW, lhsWS, rhs_of, tlen, nacc):
                    for j in range(2):
                        hs = slice(j * 128, j * 128 + 128)
                        step = min(tlen, QT)
                        for t4 in range(max(1, tlen // step)):
                            ts_ = slice(t4 * step, t4 * step + step)
                            rope_proj(
                                dest96, j, ts_,
                                [lhsW(cc, hs) for cc in range(nacc)],
                                [lhsWS(cc, hs) for cc in range(nacc)],
                                [rhs_of(cc, ts_) for cc in range(nacc)],
                                step, ts_,
                            )

                def v_tile(dest, nblk, lhs_fn, rhs_fn, nacc):
                    nc.vector.memset(dest[:, :, slice(32, 132, 33)], 1.0)
                    for tb in range(nblk):
                        ps = pp.tile([128, 128], f32, tag="p1",
                                     padded_shape=[128, QT])
                        for cc in range(nacc):
                            nc.tensor.matmul(
                                ps[:], lhs_fn(cc, tb), rhs_fn(cc),
                                start=(cc == 0), stop=(cc == nacc - 1),
                            )
                        nc.vector.tensor_copy(
                            dest[:, tb, :].rearrange(
                                "p (h c) -> p h c", h=4)[:, :, 0:32],
                            ps[:].rearrange("p (h c) -> p h c", h=4),
                        )

                def evict(jq, br, hp, ots):
                    for hi in range(2):
                        u = (br - 1) * 4 + hp * 2 + hi
                        ev = scpool.tile([33, QT], bf16, tag="ev")
                        nc.vector.tensor_copy(ev[:], ots[hi][:])
                        nc.sync.dma_start(out=d_out[jq, u], in_=ev[:])

                def branch12(br, jqs):
                    kT = k96 if br == 1 else ks96
                    vt = v1 if br == 1 else vs
                    for jq in jqs:
                        qs = slice(jq * QT, jq * QT + QT)
                        nkb = 4 * (jq + 1) if br == 1 else 4
                        for hp in range(2):
                            ots = [
                                otp.tile([33, QT], f32, name=f"ot{i}",
                                         tag=f"ot{i}", bufs=1)
                                for i in range(2)
                            ]
                            for kb in range(nkb):
                                ksl = slice(kb * 128, kb * 128 + 128)
                                sg = sgp.tile([128, 1024], f32, tag="sg")
                                # on diagonal blocks only the causally-valid
                                # q columns [128i, QT) are ever consumed
                                off = (128 * (kb - 4 * jq)
                                       if br == 1 and kb >= 4 * jq else 0)
                                for hi in range(2):
                                    h = 2 * hp + hi
                                    nc.tensor.matmul(
                                        sg[:, hi * QT + off : hi * QT + QT],
                                        kT[:, h, ksl],
                                        q96[:, h,
                                            jq * QT + off : jq * QT + QT],
                                        start=True, stop=True,
                                    )
                                pt = ptpool.tile([128, 1024], bf16, tag="pt")
                                diag = br == 1 and kb >= 4 * jq
                                if diag:
                                    # exp only the causally-valid columns;
                                    # zero the rest, then mask the triangle
                                    i = kb - 4 * jq
                                    vq = slice(128 * i, QT)
                                    sgv = sg[:].rearrange(
                                        "p (h q) -> p h q", h=2)
                                    ptv = pt[:].rearrange(
                                        "p (h q) -> p h q", h=2)
                                    if i > 0:
                                        nc.gpsimd.memset(
                                            ptv[:, :, 0 : 128 * i], 0.0)
                                    nc.scalar.activation(
                                        ptv[:, :, vq], sgv[:, :, vq],
                                        EXP, scale=SCALE,
                                    )
                                    nc.gpsimd.affine_select(
                                        out=ptv[:, :, vq], in_=ptv[:, :, vq],
                                        compare_op=GE, fill=0.0,
                                        base=0,
                                        pattern=[[0, 2], [1, QT - 128 * i]],
                                        channel_multiplier=-1,
                                    )
                                else:
                                    nc.scalar.activation(
                                        pt[:], sg[:], EXP, scale=SCALE)
                                for hi in range(2):
                                    h = 2 * hp + hi
                                    nc.tensor.matmul(
                                        ots[hi][:],
                                        vt[:, kb, 33 * h : 33 * h + 33],
                                        pt[:, hi * QT : hi * QT + QT],
                                        start=(kb == 0), stop=(kb == nkb - 1),
                                    )
                            evict(jq, br, hp, ots)

                def branch3(jqs):
                    for jq in jqs:
                        for hp in range(2):
                            ots = [
                                otp.tile([33, QT], f32, name=f"ot{i}",
                                         tag=f"ot{i}", bufs=1)
                                for i in range(2)
                            ]
                            for qcp in range(2):  # pairs of 128-q chunks
                                sg = sgp.tile([128, 1024], f32, tag="sg")
                                for qcs in range(2):
                                    qb = 4 * jq + 2 * qcp + qcs
                                    qbs = slice(qb * 128, qb * 128 + 128)
                                    for hi in range(2):
                                        h = 2 * hp + hi
                                        for ki, kb in enumerate((qb - 1, qb)):
                                            col = slice(
                                                qcs * 512 + hi * 256 + ki * 128,
                                                qcs * 512 + hi * 256 + ki * 128
                                                + 128)
                                            if kb < 0:
                                                nc.vector.memset(
                                                    sg[:, col], 0.0)
                                                continue
                                            ksl = slice(kb * 128,
                                                        kb * 128 + 128)
                                            nc.tensor.matmul(
                                                sg[:, col], kw96[:, h, ksl],
                                                q96[:, h, qbs],
                                                start=True, stop=True,
                                            )
                                pt = ptpool.tile([128, 1024], bf16, tag="pt")
                                nc.scalar.activation(pt[:], sg[:], EXP,
                                                     scale=SCALE)
                                ptv = pt[:].rearrange("p (c q) -> p c q", c=8)
                                nc.gpsimd.affine_select(  # diag: col >= row
                                    out=ptv[:, slice(1, 8, 2)],
                                    in_=ptv[:, slice(1, 8, 2)],
                                    compare_op=GE, fill=0.0, base=0,
                                    pattern=[[0, 4], [1, 128]],
                                    channel_multiplier=-1,
                                )
                                nc.gpsimd.affine_select(  # prev: row > col
                                    out=ptv[:, slice(0, 8, 2)],
                                    in_=ptv[:, slice(0, 8, 2)],
                                    compare_op=GE, fill=0.0, base=-1,
                                    pattern=[[0, 4], [-1, 128]],
                                    channel_multiplier=1,
                                )
                                for qcs in range(2):
                                    qb = 4 * jq + 2 * qcp + qcs
                                    for hi in range(2):
                                        h = 2 * hp + hi
                                        for ki, kb in enumerate((qb - 1, qb)):
                                            if kb < 0:
                                                continue
                                            col = slice(
                                                qcs * 512 + hi * 256 + ki * 128,
                                                qcs * 512 + hi * 256 + ki * 128
                                                + 128)
                                            oc = slice(
                                                (2 * qcp + qcs) * 128,
                                                (2 * qcp + qcs) * 128 + 128)
                                            nc.tensor.matmul(
                                                ots[hi][:, oc],
                                                vw[:, kb,
                                                   33 * h : 33 * h + 33],
                                                pt[:, col],
                                                start=(kb == max(qb - 1, 0)),
                                                stop=(kb == qb),
                                            )
                            evict(jq, 3, hp, ots)

                # ---- emission: q+b2 deps first (b2 is ACT-dense and can
                # start while the rest of the assembly runs on DVE) ----
                proj_nope(q96, lambda cc: s_Wqn[:],
                          lambda cc, ts_: s_cqT[:, ts_], T, 1)
                rope_all(q96, lambda cc, hs: s_Wqr[:, hs],
                         lambda cc, hs: s_WqrS[:, hs],
                         lambda cc, ts_: s_cqT[:, ts_], T, 1)
                proj_nope(ks96, lambda cc: s_Wskn[:, cc, :],
                          lambda cc, ts_: s_selT[:, cc, ts_], KEEP, 2)
                rope_all(ks96, lambda cc, hs: s_Wskr[:, cc, hs],
                         lambda cc, hs: s_WskrS[:, cc, hs],
                         lambda cc, ts_: s_selT[:, cc, ts_], KEEP, 2)
                v_tile(
                    vs, 4,
                    lambda cc, tb: s_selT[:, cc, tb * 128 : tb * 128 + 128],
                    lambda cc: s_Wsv[:, cc, :], 2,
                )
                branch12(2, range(NJQ))
                proj_nope(k96, lambda cc: s_Wkn[:],
                          lambda cc, ts_: s_ckvT[:, ts_], T, 1)
                v_tile(
                    v1, NKB,
                    lambda cc, tb: s_ckvT[:, tb * 128 : tb * 128 + 128],
                    lambda cc: s_Wv[:], 1,
                )
                branch12(1, range(NJQ - 1))
                proj_nope(kw96, lambda cc: s_Wwkn[:, cc, :],
                          lambda cc, ts_: s_xT[:, cc, ts_], T, 2)
                rope_all(kw96, lambda cc, hs: s_Wwkr[:, cc, hs],
                         lambda cc, hs: s_WwkrS[:, cc, hs],
                         lambda cc, ts_: s_xT[:, cc, ts_], T, 2)
                v_tile(
                    vw, NKB,
                    lambda cc, tb: s_xT[:, cc, tb * 128 : tb * 128 + 128],
                    lambda cc: s_Wwv[:, cc, :], 2,
                )
                branch3(range(NJQ))
                branch12(1, [NJQ - 1])

    if legalize:
        _legalize_pe_waits(nc, mybir)
    return nc


def _legalize_pe_waits(nc, mybir):
    """This walrus build encodes at most ONE sync-wait per compute
    instruction, but Tile emits up to 3. Split excess waits into standalone
    same-engine InstEventSemaphore waits placed immediately before the
    instruction (program point unchanged, so no deadlock risk)."""
    exempt = ("InstEventSemaphore", "InstNoOp",
              "InstUnconditionalBranch", "InstCall", "InstISA")
    for f in nc.m.functions:
        for bb in f.blocks:
            out = []
            changed = False
            for inst in bb.instructions:
                si = inst.sync_info
                tname = type(inst).__name__
                if si is not None and len(si.on_wait) > 1 and tname not in exempt:
                    for k, w in enumerate(si.on_wait[:-1]):
                        out.append(mybir.InstEventSemaphore(
                            name=f"{inst.name}-wsplit{k}",
                            engine=inst.engine,
                            ins=[], outs=[],
                            sync_info=mybir.SyncInfo(
                                on_wait=[w], on_update=[]),
                        ))
                    inst.sync_info = mybir.SyncInfo(
                        on_wait=[si.on_wait[-1]],
                        on_update=list(si.on_update),
                    )
                    changed = True
                out.append(inst)
            if changed:
                bb.instructions = out


def _get_nc():
    if "nc" not in _CACHE:
        _CACHE["nc"] = _build_bass()
    return _CACHE["nc"]


# ---------------------------------------------------------------------------
# host orchestration
# ---------------------------------------------------------------------------

def _prep_in_maps(inputs):
    x = np.asarray(inputs["x"], np.float32)
    cos, sin = _freqs(T)
    c128, s128 = _rope_tables()

    Wqn_r = np.asarray(inputs["Wq_nope"], np.float32).reshape(96, N_HEAD, NOPE)
    Wqr_r = np.asarray(inputs["Wq_rope"], np.float32).reshape(96, N_HEAD, ROPE)
    Wkn_r = np.asarray(inputs["Wk_nope"], np.float32).reshape(32, N_HEAD, NOPE)
    Wv_r = np.asarray(inputs["Wv"], np.float32).reshape(32, N_HEAD, VDIM)
    Wsk_r = np.asarray(inputs["Wsel_k"], np.float32).reshape(C, N_HEAD, HD)
    Wsv_r = np.asarray(inputs["Wsel_v"], np.float32).reshape(C, N_HEAD, VDIM)
    Wwk_r = np.asarray(inputs["Wwin_k"], np.float32).reshape(C, N_HEAD, HD)
    Wwv_r = np.asarray(inputs["Wwin_v"], np.float32).reshape(C, N_HEAD, VDIM)
    Wp = np.asarray(inputs["Wproj"], np.float32)

    per_batch = []
    gates = []
    for b in range(B):
        xb = x[b]
        cq = _rms(xb @ np.asarray(inputs["Wcq"], np.float32),
                  np.asarray(inputs["q_norm_w"], np.float32))
        ckv = _rms(xb @ np.asarray(inputs["Wckv"], np.float32),
                   np.asarray(inputs["kv_norm_w"], np.float32))
        kr = _rope_host(
            (xb @ np.asarray(inputs["Wk_rope"], np.float32)) / N_HEAD, cos, sin
        )
        glog = (xb @ np.asarray(inputs["Wgate"], np.float32)).mean(0)
        g = np.exp(glog - glog.max())
        g = g / g.sum()
        scores = (xb @ np.asarray(inputs["W_imp"], np.float32))[:, 0]
        idx = np.sort(np.argpartition(-scores, KEEP - 1)[:KEEP])
        sel = xb[idx]
        per_batch.append(dict(
            xT=np.ascontiguousarray(xb.T.reshape(2, 128, T)).astype(BF),
            cqT=np.ascontiguousarray(cq.T).astype(BF),
            ckvT=np.ascontiguousarray(ckv.T).astype(BF),
            krT=np.ascontiguousarray(kr.T).astype(BF),
            selT=np.ascontiguousarray(sel.T.reshape(2, 128, KEEP)).astype(BF),
        ))
        gates.append(g.astype(np.float32))

    in_maps = []
    for core in range(N_CORES):
        b, hg = divmod(core, N_CORES // B)
        hsl = slice(hg * HPC, hg * HPC + HPC)
        m = dict(per_batch[b])

        def hw(w):  # [cin, 4, d] -> bf16 [cin, 4*d]
            return np.ascontiguousarray(
                w.reshape(w.shape[0], -1)
            ).astype(BF)

        m["Wqn"] = hw(Wqn_r[:, hsl])
        m["Wqr"] = hw(Wqr_r[:, hsl])
        m["WqrS"] = hw(_swap_cols(Wqr_r[:, hsl]))
        m["Wkn"] = hw(Wkn_r[:, hsl])
        m["Wv"] = hw(Wv_r[:, hsl])
        m["Wskn"] = hw(Wsk_r[:, hsl, :NOPE]).reshape(2, 128, 128)
        m["Wskr"] = hw(Wsk_r[:, hsl, NOPE:]).reshape(2, 128, 256)
        m["WskrS"] = hw(_swap_cols(Wsk_r[:, hsl, NOPE:])).reshape(2, 128, 256)
        m["Wsv"] = hw(Wsv_r[:, hsl]).reshape(2, 128, 128)
        m["Wwkn"] = hw(Wwk_r[:, hsl, :NOPE]).reshape(2, 128, 128)
        m["Wwkr"] = hw(Wwk_r[:, hsl, NOPE:]).reshape(2, 128, 256)
        m["WwkrS"] = hw(_swap_cols(Wwk_r[:, hsl, NOPE:])).reshape(2, 128, 256)
        m["Wwv"] = hw(Wwv_r[:, hsl]).reshape(2, 128, 128)
        m["cosT"] = c128
        m["sinT"] = s128
        in_maps.append(m)
    return in_maps, gates, Wp


def _epilogue_core(raw, g, Wp_hg):
    """raw [NJQ, 12, 33, QT] bf16 -> normalized, gated, projected [T, C]."""
    r = np.asarray(raw, np.float32)
    # [12, 33, T]
    r = r.transpose(1, 2, 0, 3).reshape(12, 33, T)
    vals = r[:, :32, :]                      # [12, 32, T]
    z = r[:, 32, :]                          # [12, T]
    o = vals / z[:, None, :]                 # normalized per (branch,head)
    o = o.reshape(3, 4, 32, T)
    acc = (g[:, None, None, None] * o).sum(0)  # [4, 32, T]
    acc = acc.reshape(128, T)
    return acc.T @ Wp_hg                     # [T, C]


def _run(inputs, trace=False):
    from concourse.bass_utils import run_bass_kernel_spmd

    nc = _get_nc()
    in_maps, gates, Wp = _prep_in_maps(inputs)
    res = run_bass_kernel_spmd(nc, in_maps, list(range(N_CORES)), trace=trace)
    out = np.zeros((B, T, C), np.float32)
    for core in range(N_CORES):
        b, hg = divmod(core, N_CORES // B)
        out[b] += _epilogue_core(
            res.results[core]["outT"], gates[b],
            Wp[hg * 128 : hg * 128 + 128],
        )
    return out, res


def kernel(**inputs):
    out, _ = _run(inputs, trace=False)
    return out



# revision 17
# speedup vs baseline: 2.8342x; 2.8342x over previous
"""nn_Attn_9715216024104 — sparse attention (MLA + top-k select + sliding window).

Sharding: 8 cores = 2 batches x 4 head-groups (4 heads each). Each core runs
one Bass/Tile kernel computing its 4 heads' three attention branches
(S^T layout, exp softmax without max-subtraction — scores are <0.5 — with
ones-column-folded Z rows in the PV matmul).

This revision minimizes per-call host<->device traffic (the axon tunnel runs
at ~45 MB/s, so bytes dominate the wall clock):
- Inputs are packed into 3 DRAM params (blob[128,NB], b96, b32) instead of 20.
- cq/ckv RMS-norm and the shared roped kr are computed ON DEVICE from x
  (norm weights folded into downstream projections on the host; rms factor
  via ones-matmul column reduce + Sqrt activation + DVE reciprocal + rank-1
  f32 matmul partition-broadcast).
- Rope cos/sin tables ship compact as [32,T] and are replicated/sign-folded
  on device; "swapped" rope projection weights are built on device by
  column-half swaps instead of being shipped twice.
- The epilogue (divide by Z, gate, sum the 3 branches) runs ON DEVICE:
  branch gates are folded into the V projection weights on the host, evicts
  normalize by the PSUM Z row (DVE reciprocal + rank-1 broadcast) and
  accumulate into a f32 [128,T] tile; output is a single bf16 [128,T] per
  core. Host only applies Wproj per head-group and sums.
- kernel.py also enables JAX's persistent compilation cache so repeated
  run_bass_kernel_spmd calls skip the per-call XLA/NEFF rebuild.

Device layout notes:
- All matmul operands bf16 (rank-1 Z/rms broadcasts use f32); PSUM f32.
- Attention uses S^T tiles [k=128, q] so P^T feeds the PV matmul directly;
  V tiles carry a ones column so the PV matmul also produces Z rows.
- Rope is applied via duplicated "swapped" projection weights:
  rope(x) = x * cos + swap(x) * sgn*sin, with swap folded into a second
  matmul, so DVE only does 2 muls + 1 add.
- Causal / sliding-window masking is done on GPSIMD (affine_select zeroing
  of P^T after exp), keeping TensorE/ACT free of mask work.
"""

import math

import numpy as np
import ml_dtypes

try:  # persistent XLA compilation cache: the per-call jit is a fresh closure
    import jax

    jax.config.update("jax_compilation_cache_dir", "/tmp/jaxcache")
    jax.config.update("jax_persistent_cache_min_compile_time_secs", 0.0)
    jax.config.update("jax_persistent_cache_min_entry_size_bytes", 0)
except Exception:
    pass

BF = ml_dtypes.bfloat16

N_HEAD = 16
NOPE = 32
ROPE = 64
VDIM = 32
HD = NOPE + ROPE  # 96
WINDOW = 128
KEEP = 512
EPS = 1e-6
N_CORES = 8
HPC = 4  # heads per core
B, T, C = 2, 2048, 256
QT = 512  # q tile (free dim)
NJQ = T // QT  # 4 q tiles
NKB = T // 128  # 16 k blocks
SCALE = 1.0 / math.sqrt(HD)

# blob column offsets (all bf16, [128, NB])
XT = 0  # x^T, 2 cin chunks       [128, 2*T]
SEL = XT + 2 * T  # sel^T, 2 cin chunks     [128, 2*KEEP]
WSKN = SEL + 2 * KEEP  # Wsel_k nope, 2 chunks   [128, 2*128]
WSKR = WSKN + 256  # Wsel_k rope, 2 chunks   [128, 2*256]
WSV = WSKR + 512  # Wsel_v (gated), 2 chunks [128, 2*128]
WWKN = WSV + 256  # Wwin_k nope             [128, 2*128]
WWKR = WWKN + 256  # Wwin_k rope             [128, 2*256]
WWV = WWKR + 512  # Wwin_v (gated)          [128, 2*128]
WCQ = WWV + 256  # Wcq                     [128, 2*96]
WCKV = WCQ + 192  # Wckv                    [128, 2*32]
WKR = WCKV + 64  # Wk_rope/N_HEAD          [128, 2*64]
NB = WKR + 128

# b96 columns ([96, N96])
WQN = 0  # Wq_nope (norm-folded)  [96, 128]
WQR = 128  # Wq_rope (norm-folded)  [96, 256]
N96 = 384

# b32 columns ([32, N32])
C32 = 0  # cos^T [32, T]
S32 = T  # sin^T [32, T]
WKN = 2 * T  # Wk_nope (norm-folded) [32, 128]
WV = 2 * T + 128  # Wv (norm- and gate-folded) [32, 128]
N32 = 2 * T + 256

_CACHE = {}


# ---------------------------------------------------------------------------
# host-side helpers
# ---------------------------------------------------------------------------

def _freqs(t):
    f = 1.0 / 1e4 ** (np.arange(0, ROPE, 2, dtype=np.float32) / ROPE)
    ang = np.outer(np.arange(t, dtype=np.float32), f)
    return np.cos(ang).astype(np.float32), np.sin(ang).astype(np.float32)


# ---------------------------------------------------------------------------
# bass program (built once; identical for all 8 cores)
# ---------------------------------------------------------------------------

def _build_bass(legalize=True):
    import concourse.bass as bass
    import concourse.mybir as mybir
    import concourse.tile as tile

    f32 = mybir.dt.float32
    bf16 = mybir.dt.bfloat16
    EXP = mybir.ActivationFunctionType.Exp
    SQRT = mybir.ActivationFunctionType.Sqrt
    GE = mybir.AluOpType.is_ge

    nc = bass.Bass(target_bir_lowering=False, debug=False)

    d_blob = nc.declare_dram_parameter("blob", [128, NB], bf16, isOutput=False)
    d_b96 = nc.declare_dram_parameter("b96", [96, N96], bf16, isOutput=False)
    d_b32 = nc.declare_dram_parameter("b32", [32, N32], bf16, isOutput=False)
    d_out = nc.declare_dram_parameter("outT", [128, T], bf16, isOutput=True)

    def asl(base, s):  # absolute blob column slice
        return slice(base + s.start, base + s.stop)

    with tile.TileContext(nc) as tc:
        with (
            tc.tile_pool(name="const", bufs=1) as cpool,
            tc.tile_pool(name="big", bufs=1) as bpool,
            tc.tile_pool(name="pt", bufs=3) as ptpool,
            tc.tile_pool(name="sc", bufs=4) as scpool,
        ):
            _dma_engines = [nc.sync, nc.gpsimd, nc.scalar]
            _dma_rr = [0]

            def _dma(out, in_):
                eng = _dma_engines[_dma_rr[0] % len(_dma_engines)]
                _dma_rr[0] += 1
                eng.dma_start(out=out, in_=in_)

            s_blob = cpool.tile([128, NB], bf16, name="blob", tag="blob")
            s_b96 = cpool.tile([96, N96], bf16, name="b96", tag="b96")
            s_b32 = cpool.tile([32, N32], bf16, name="b32", tag="b32")

            # small/early-needed first; big x/sel tensors split for overlap
            _dma(s_b32[:, :], d_b32[:, :])
            _dma(s_b96[:, :], d_b96[:, :])
            _dma(s_blob[:, WSKN:NB], d_blob[:, WSKN:NB])
            _dma(s_blob[:, XT : XT + T], d_blob[:, XT : XT + T])
            _dma(s_blob[:, XT + T : XT + 2 * T], d_blob[:, XT + T : XT + 2 * T])
            _dma(s_blob[:, SEL : SEL + 2 * KEEP], d_blob[:, SEL : SEL + 2 * KEEP])

            def xT_ap(cc, ts_):
                return s_blob[:, asl(XT + cc * T, ts_)]

            # ---- rope tables [128, T] from compact [32, T] + sign folding ----
            c128 = cpool.tile([128, T], bf16, name="c128", tag="c128")
            s128 = cpool.tile([128, T], bf16, name="s128", tag="s128")
            for blk in range(4):
                rs = slice(32 * blk, 32 * blk + 32)
                nc.scalar.copy(c128[rs, :], s_b32[:, C32 : C32 + T])
                if blk % 2 == 0:
                    nc.vector.tensor_scalar_mul(
                        s128[rs, :], s_b32[:, S32 : S32 + T], -1.0)
                else:
                    nc.gpsimd.tensor_copy(s128[rs, :], s_b32[:, S32 : S32 + T])

            # ---- swapped rope weights built on device (column-half swap) ----
            wqrS = cpool.tile([96, 256], bf16, tag="wqrS")
            wskrS = cpool.tile([128, 2, 256], bf16, tag="wskrS")
            wwkrS = cpool.tile([128, 2, 256], bf16, tag="wwkrS")
            wkrS = cpool.tile([128, 2, 64], bf16, tag="wkrS")
            _sw_rr = [0]

            def _sweng():
                eng = (nc.vector, nc.gpsimd, nc.scalar)[_sw_rr[0] % 3]
                _sw_rr[0] += 1
                return eng

            def swap_into(dst_ap, src_ap, nh):
                # both viewed [P, nh, 2, 32]; swap axis-2 halves
                dv = dst_ap.rearrange("p (h two c) -> p h two c", two=2, c=32)
                sv = src_ap.rearrange("p (h two c) -> p h two c", two=2, c=32)
                for half in range(2):
                    eng = _sweng()
                    (eng.tensor_copy if eng is not nc.scalar else eng.copy)(
                        dv[:, :, half, :], sv[:, :, 1 - half, :])

            swap_into(wqrS[:, :], s_b96[:, WQR : WQR + 256], 4)
            for cc in range(2):
                swap_into(wskrS[:, cc, :],
                          s_blob[:, WSKR + cc * 256 : WSKR + cc * 256 + 256], 4)
                swap_into(wwkrS[:, cc, :],
                          s_blob[:, WWKR + cc * 256 : WWKR + cc * 256 + 256], 4)
                swap_into(wkrS[:, cc, :],
                          s_blob[:, WKR + cc * 64 : WKR + cc * 64 + 64], 1)

            # ---- constants for reductions/broadcasts ----
            ones96c = cpool.tile([96, 1], bf16, tag="o96c")
            ones32c = cpool.tile([32, 1], bf16, tag="o32c")
            ones96r = cpool.tile([1, 96], f32, tag="o96r")
            ones32r = cpool.tile([1, 32], f32, tag="o32r")
            epsc = cpool.tile([1, 1], f32, tag="epsc")
            nc.vector.memset(ones96c[:, :], 1.0)
            nc.vector.memset(ones32c[:, :], 1.0)
            nc.vector.memset(ones96r[:, :], 1.0)
            nc.vector.memset(ones32r[:, :], 1.0)
            nc.vector.memset(epsc[:, :], EPS)

            # ---- assembled per-head [96, h, T] q/k layouts ----
            cqT = bpool.tile([96, T], bf16)   # rms-normalized cq^T
            ckvT = bpool.tile([32, T], bf16)  # rms-normalized ckv^T
            q96 = bpool.tile([96, 4, T], bf16)
            k96 = bpool.tile([96, 4, T], bf16)     # branch 1 (kn | shared kr)
            ks96 = bpool.tile([96, 4, KEEP], bf16)  # branch 2
            kw96 = bpool.tile([96, 4, T], bf16)    # branch 3
            v1 = bpool.tile([128, NKB, 132], bf16)
            vs = bpool.tile([128, 4, 132], bf16)
            vw = bpool.tile([128, NKB, 132], bf16)
            acc = bpool.tile([128, T], f32)        # gated, normalized output
            outb = bpool.tile([128, T], bf16)

            with (
                tc.tile_pool(name="pp", bufs=2, space=bass.MemorySpace.PSUM) as pp,
                tc.tile_pool(name="sgp", bufs=2, space=bass.MemorySpace.PSUM) as sgp,
                tc.tile_pool(name="otp", bufs=2, space=bass.MemorySpace.PSUM) as otp,
            ):
                def rmsnorm_proj(dst, p, wcol, wwid, ones_col, ones_row, inv_n):
                    """dst[p, T] <- rms-normalized W^T x^T (tokens on free dim).
                    rms factor: ones-matmul column sum of squares -> Sqrt ACT
                    -> DVE reciprocal -> rank-1 f32 matmul broadcast."""
                    for t4 in range(NJQ):
                        ts_ = slice(t4 * QT, t4 * QT + QT)
                        ps = pp.tile([128, QT], f32, tag="p1",
                                     padded_shape=[128, QT])
                        for cc in range(2):
                            nc.tensor.matmul(
                                ps[0:p, :],
                                s_blob[:, wcol + cc * wwid : wcol + (cc + 1) * wwid],
                                xT_ap(cc, ts_), start=(cc == 0), stop=(cc == 1),
                            )
                        pre = scpool.tile([p, QT], bf16, tag="pre")
                        nc.scalar.copy(pre[:, :], ps[0:p, :])
                        sq = scpool.tile([p, QT], bf16, tag="sq")
                        nc.vector.tensor_mul(sq[:, :], pre[:, :], pre[:, :])
                        ps2 = pp.tile([1, QT], f32, tag="p1",
                                      padded_shape=[128, QT])
                        nc.tensor.matmul(ps2[:, :], ones_col[:, :], sq[:, :],
                                         start=True, stop=True)
                        srow = scpool.tile([1, QT], f32, tag="srow", bufs=2)
                        nc.scalar.activation(srow[:, :], ps2[:, :], SQRT,
                                             bias=epsc[:, :], scale=inv_n)
                        rin = scpool.tile([1, QT], f32, tag="rin", bufs=2)
                        nc.vector.reciprocal(rin[:, :], srow[:, :])
                        bc = pp.tile([128, QT], f32, tag="p1",
                                     padded_shape=[128, QT])
                        nc.tensor.matmul(bc[0:p, :], ones_row[:, 0:p],
                                         rin[:, :], start=True, stop=True)
                        nc.vector.tensor_mul(dst[:, ts_], pre[:, :], bc[0:p, :])

                def kr_build():
                    """k96[0:64, h, :] <- rope((x @ Wk_rope)/N_HEAD), all heads."""
                    for t4 in range(NJQ):
                        ts_ = slice(t4 * QT, t4 * QT + QT)
                        pr = pp.tile([128, QT], f32, tag="p1",
                                     padded_shape=[128, QT])
                        psw = pp.tile([128, QT], f32, tag="p1",
                                      padded_shape=[128, QT])
                        for cc in range(2):
                            nc.tensor.matmul(
                                pr[0:64, :],
                                s_blob[:, WKR + cc * 64 : WKR + cc * 64 + 64],
                                xT_ap(cc, ts_), start=(cc == 0), stop=(cc == 1),
                            )
                        for cc in range(2):
                            nc.tensor.matmul(
                                psw[0:64, :], wkrS[:, cc, :], xT_ap(cc, ts_),
                                start=(cc == 0), stop=(cc == 1),
                            )
                        t1 = scpool.tile([64, QT], bf16, tag="rt1")
                        t2 = scpool.tile([64, QT], bf16, tag="rt2")
                        nc.vector.tensor_mul(t1[:, :], pr[0:64, :],
                                             c128[0:64, ts_])
                        nc.vector.tensor_mul(t2[:, :], psw[0:64, :],
                                             s128[0:64, ts_])
                        nc.gpsimd.tensor_add(k96[0:64, 0, ts_], t1[:, :],
                                             t2[:, :])
                        for h in range(1, 4):
                            nc.scalar.copy(k96[0:64, h, ts_], k96[0:64, 0, ts_])

                def proj_nope(dest96, lhsW, rhs_of, tlen, nacc, eng=None):
                    """4-head nope projection, split per head into
                    dest96[64:96, h, ts]."""
                    step = min(tlen, QT)
                    for t4 in range(max(1, tlen // step)):
                        ts_ = slice(t4 * step, t4 * step + step)
                        ps = pp.tile([128, step], f32, tag="p1",
                                     padded_shape=[128, QT])
                        for cc in range(nacc):
                            nc.tensor.matmul(
                                ps[:], lhsW(cc), rhs_of(cc, ts_),
                                start=(cc == 0), stop=(cc == nacc - 1),
                            )
                        for h in range(4):
                            if eng == "act":
                                nc.scalar.copy(
                                    dest96[64:96, h, ts_],
                                    ps[32 * h : 32 * h + 32, :],
                                )
                            else:
                                nc.vector.tensor_copy(
                                    dest96[64:96, h, ts_],
                                    ps[32 * h : 32 * h + 32, :],
                                )

                def rope_proj(dest96, hpair, cos_sl, lhs_raw, lhs_sw,
                              rhs_list, tlen, ts_):
                    """Rope for one head-pair chunk; writes per-head rows
                    dest96[0:64, h, ts]."""
                    pr = pp.tile([128, tlen], f32, tag="p1",
                                 padded_shape=[128, QT])
                    psw = pp.tile([128, tlen], f32, tag="p1",
                                  padded_shape=[128, QT])
                    ncc = len(rhs_list)
                    for cc, rhs in enumerate(rhs_list):
                        nc.tensor.matmul(
                            pr[:], lhs_raw[cc], rhs,
                            start=(cc == 0), stop=(cc == ncc - 1),
                        )
                    for cc, rhs in enumerate(rhs_list):
                        nc.tensor.matmul(
                            psw[:], lhs_sw[cc], rhs,
                            start=(cc == 0), stop=(cc == ncc - 1),
                        )
                    t1 = scpool.tile([128, tlen], bf16, tag="rt1")
                    t2 = scpool.tile([128, tlen], bf16, tag="rt2")
                    nc.vector.tensor_mul(t1[:], pr[:], c128[:, cos_sl])
                    nc.vector.tensor_mul(t2[:], psw[:], s128[:, cos_sl])
                    for hi in range(2):
                        h = 2 * hpair + hi
                        hr = slice(64 * hi, 64 * hi + 64)
                        nc.gpsimd.tensor_add(
                            dest96[0:64, h, ts_], t1[hr, :], t2[hr, :]
                        )

                def rope_all(dest96, lhsW, lhsWS, rhs_of, tlen, nacc):
                    for j in range(2):
                        hs = slice(j * 128, j * 128 + 128)
                        step = min(tlen, QT)
                        for t4 in range(max(1, tlen // step)):
                            ts_ = slice(t4 * step, t4 * step + step)
                            rope_proj(
                                dest96, j, ts_,
                                [lhsW(cc, hs) for cc in range(nacc)],
                                [lhsWS(cc, hs) for cc in range(nacc)],
                                [rhs_of(cc, ts_) for cc in range(nacc)],
                                step, ts_,
                            )

                def v_tile(dest, nblk, lhs_fn, rhs_fn, nacc):
                    nc.vector.memset(dest[:, :, slice(32, 132, 33)], 1.0)
                    for tb in range(nblk):
                        ps = pp.tile([128, 128], f32, tag="p1",
                                     padded_shape=[128, QT])
                        for cc in range(nacc):
                            nc.tensor.matmul(
                                ps[:], lhs_fn(cc, tb), rhs_fn(cc),
                                start=(cc == 0), stop=(cc == nacc - 1),
                            )
                        nc.vector.tensor_copy(
                            dest[:, tb, :].rearrange(
                                "p (h c) -> p h c", h=4)[:, :, 0:32],
                            ps[:].rearrange("p (h c) -> p h c", h=4),
                        )

                def evict(jq, br, hp, ots):
                    """Normalize by the PSUM Z row and accumulate (gates are
                    folded into the V weights host-side)."""
                    oc = slice(jq * QT, jq * QT + QT)
                    for hi in range(2):
                        h = 2 * hp + hi
                        zi = scpool.tile([1, QT], f32, tag="zi", bufs=2)
                        nc.vector.reciprocal(zi[:, :], ots[hi][32:33, :])
                        zb = pp.tile([32, QT], f32, tag="p1",
                                     padded_shape=[128, QT])
                        nc.tensor.matmul(zb[:, :], ones32r[:, :], zi[:, :],
                                         start=True, stop=True)
                        # two PSUM srcs in one DVE op are illegal; stage zb
                        zbs = scpool.tile([32, QT], f32, tag="zbs", bufs=2)
                        nc.vector.tensor_copy(zbs[:, :], zb[:, :])
                        hr = slice(32 * h, 32 * h + 32)
                        ar = acc[hr, oc]
                        if br == 2:  # first writer of this acc region
                            nc.vector.tensor_mul(ar, ots[hi][0:32, :],
                                                 zbs[:, :])
                        else:
                            # tm band matches acc's partitions: walrus wants
                            # TT *inputs* on the same start partition
                            tm = scpool.tile([128, QT], f32, tag="tm", bufs=2)
                            nc.vector.tensor_mul(tm[hr, :], ots[hi][0:32, :],
                                                 zbs[:, :])
                            nc.gpsimd.tensor_add(ar, ar, tm[hr, :])

                def branch12(br, jqs):
                    kT = k96 if br == 1 else ks96
                    vt = v1 if br == 1 else vs
                    for jq in jqs:
                        nkb = 4 * (jq + 1) if br == 1 else 4
                        for hp in range(2):
                            ots = [
                                otp.tile([33, QT], f32, name=f"ot{i}",
                                         tag=f"ot{i}", bufs=1)
                                for i in range(2)
                            ]
                            for kb in range(nkb):
                                ksl = slice(kb * 128, kb * 128 + 128)
                                sg = sgp.tile([128, 1024], f32, tag="sg")
                                # on diagonal blocks only the causally-valid
                                # q columns [128i, QT) are ever consumed
                                off = (128 * (kb - 4 * jq)
                                       if br == 1 and kb >= 4 * jq else 0)
                                for hi in range(2):
                                    h = 2 * hp + hi
                                    nc.tensor.matmul(
                                        sg[:, hi * QT + off : hi * QT + QT],
                                        kT[:, h, ksl],
                                        q96[:, h,
                                            jq * QT + off : jq * QT + QT],
                                        start=True, stop=True,
                                    )
                                pt = ptpool.tile([128, 1024], bf16, tag="pt")
                                diag = br == 1 and kb >= 4 * jq
                                if diag:
                                    # exp only the causally-valid columns;
                                    # zero the rest, then mask the triangle
                                    i = kb - 4 * jq
                                    vq = slice(128 * i, QT)
                                    sgv = sg[:].rearrange(
                                        "p (h q) -> p h q", h=2)
                                    ptv = pt[:].rearrange(
                                        "p (h q) -> p h q", h=2)
                                    if i > 0:
                                        nc.gpsimd.memset(
                                            ptv[:, :, 0 : 128 * i], 0.0)
                                    nc.scalar.activation(
                                        ptv[:, :, vq], sgv[:, :, vq],
                                        EXP, scale=SCALE,
                                    )
                                    nc.gpsimd.affine_select(
                                        out=ptv[:, :, vq], in_=ptv[:, :, vq],
                                        compare_op=GE, fill=0.0,
                                        base=0,
                                        pattern=[[0, 2], [1, QT - 128 * i]],
                                        channel_multiplier=-1,
                                    )
                                else:
                                    nc.scalar.activation(
                                        pt[:], sg[:], EXP, scale=SCALE)
                                for hi in range(2):
                                    h = 2 * hp + hi
                                    nc.tensor.matmul(
                                        ots[hi][:],
                                        vt[:, kb, 33 * h : 33 * h + 33],
                                        pt[:, hi * QT : hi * QT + QT],
                                        start=(kb == 0), stop=(kb == nkb - 1),
                                    )
                            evict(jq, br, hp, ots)

                def branch3(jqs):
                    for jq in jqs:
                        for hp in range(2):
                            ots = [
                                otp.tile([33, QT], f32, name=f"ot{i}",
                                         tag=f"ot{i}", bufs=1)
                                for i in range(2)
                            ]
                            for qcp in range(2):  # pairs of 128-q chunks
                                sg = sgp.tile([128, 1024], f32, tag="sg")
                                for qcs in range(2):
                                    qb = 4 * jq + 2 * qcp + qcs
                                    qbs = slice(qb * 128, qb * 128 + 128)
                                    for hi in range(2):
                                        h = 2 * hp + hi
                                        for ki, kb in enumerate((qb - 1, qb)):
                                            col = slice(
                                                qcs * 512 + hi * 256 + ki * 128,
                                                qcs * 512 + hi * 256 + ki * 128
                                                + 128)
                                            if kb < 0:
                                                nc.vector.memset(
                                                    sg[:, col], 0.0)
                                                continue
                                            ksl = slice(kb * 128,
                                                        kb * 128 + 128)
                                            nc.tensor.matmul(
                                                sg[:, col], kw96[:, h, ksl],
                                                q96[:, h, qbs],
                                                start=True, stop=True,
                                            )
                                pt = ptpool.tile([128, 1024], bf16, tag="pt")
                                nc.scalar.activation(pt[:], sg[:], EXP,
                                                     scale=SCALE)
                                ptv = pt[:].rearrange("p (c q) -> p c q", c=8)
                                nc.gpsimd.affine_select(  # diag: col >= row
                                    out=ptv[:, slice(1, 8, 2)],
                                    in_=ptv[:, slice(1, 8, 2)],
                                    compare_op=GE, fill=0.0, base=0,
                                    pattern=[[0, 4], [1, 128]],
                                    channel_multiplier=-1,
                                )
                                nc.gpsimd.affine_select(  # prev: row > col
                                    out=ptv[:, slice(0, 8, 2)],
                                    in_=ptv[:, slice(0, 8, 2)],
                                    compare_op=GE, fill=0.0, base=-1,
                                    pattern=[[0, 4], [-1, 128]],
                                    channel_multiplier=1,
                                )
                                for qcs in range(2):
                                    qb = 4 * jq + 2 * qcp + qcs
                                    for hi in range(2):
                                        h = 2 * hp + hi
                                        for ki, kb in enumerate((qb - 1, qb)):
                                            if kb < 0:
                                                continue
                                            col = slice(
                                                qcs * 512 + hi * 256 + ki * 128,
                                                qcs * 512 + hi * 256 + ki * 128
                                                + 128)
                                            oc = slice(
                                                (2 * qcp + qcs) * 128,
                                                (2 * qcp + qcs) * 128 + 128)
                                            nc.tensor.matmul(
                                                ots[hi][:, oc],
                                                vw[:, kb,
                                                   33 * h : 33 * h + 33],
                                                pt[:, col],
                                                start=(kb == max(qb - 1, 0)),
                                                stop=(kb == qb),
                                            )
                            evict(jq, 3, hp, ots)

                # ---- emission: q+b2 deps first (b2 is ACT-dense and can
                # start while the rest of the assembly runs on DVE) ----
                rmsnorm_proj(cqT, 96, WCQ, 96, ones96c, ones96r, 1.0 / 96)
                proj_nope(q96, lambda cc: s_b96[:, WQN : WQN + 128],
                          lambda cc, ts_: cqT[:, ts_], T, 1)
                rope_all(q96,
                         lambda cc, hs: s_b96[:, asl(WQR, hs)],
                         lambda cc, hs: wqrS[:, hs],
                         lambda cc, ts_: cqT[:, ts_], T, 1)
                proj_nope(ks96,
                          lambda cc: s_blob[:, WSKN + cc * 128 : WSKN + cc * 128 + 128],
                          lambda cc, ts_: s_blob[:, asl(SEL + cc * KEEP, ts_)],
                          KEEP, 2)
                rope_all(ks96,
                         lambda cc, hs: s_blob[:, asl(WSKR + cc * 256, hs)],
                         lambda cc, hs: wskrS[:, cc, hs],
                         lambda cc, ts_: s_blob[:, asl(SEL + cc * KEEP, ts_)],
                         KEEP, 2)
                v_tile(
                    vs, 4,
                    lambda cc, tb: s_blob[:, SEL + cc * KEEP + tb * 128 :
                                          SEL + cc * KEEP + tb * 128 + 128],
                    lambda cc: s_blob[:, WSV + cc * 128 : WSV + cc * 128 + 128],
                    2,
                )
                branch12(2, range(NJQ))
                rmsnorm_proj(ckvT, 32, WCKV, 32, ones32c, ones32r, 1.0 / 32)
                proj_nope(k96, lambda cc: s_b32[:, WKN : WKN + 128],
                          lambda cc, ts_: ckvT[:, ts_], T, 1)
                v_tile(
                    v1, NKB,
                    lambda cc, tb: ckvT[:, tb * 128 : tb * 128 + 128],
                    lambda cc: s_b32[:, WV : WV + 128], 1,
                )
                kr_build()
                branch12(1, range(NJQ - 1))
                proj_nope(kw96,
                          lambda cc: s_blob[:, WWKN + cc * 128 : WWKN + cc * 128 + 128],
                          lambda cc, ts_: xT_ap(cc, ts_), T, 2)
                rope_all(kw96,
                         lambda cc, hs: s_blob[:, asl(WWKR + cc * 256, hs)],
                         lambda cc, hs: wwkrS[:, cc, hs],
                         lambda cc, ts_: xT_ap(cc, ts_), T, 2)
                v_tile(
                    vw, NKB,
                    lambda cc, tb: xT_ap(cc, slice(tb * 128, tb * 128 + 128)),
                    lambda cc: s_blob[:, WWV + cc * 128 : WWV + cc * 128 + 128],
                    2,
                )
                branch3(range(NJQ))
                branch12(1, [NJQ - 1])

                # ---- final: bf16 cast + output DMA (split by jq) ----
                for jq in range(NJQ):
                    oc = slice(jq * QT, jq * QT + QT)
                    nc.scalar.copy(outb[:, oc], acc[:, oc])
                    _dma(d_out[:, oc], outb[:, oc])

    if legalize:
        _legalize_pe_waits(nc, mybir)
    return nc


def _legalize_pe_waits(nc, mybir):
    """This walrus build encodes at most ONE sync-wait per compute
    instruction, but Tile emits up to 3. Split excess waits into standalone
    same-engine InstEventSemaphore waits placed immediately before the
    instruction (program point unchanged, so no deadlock risk)."""
    exempt = ("InstEventSemaphore", "InstNoOp",
              "InstUnconditionalBranch", "InstCall", "InstISA")
    for f in nc.m.functions:
        for bb in f.blocks:
            out = []
            changed = False
            for inst in bb.instructions:
                si = inst.sync_info
                tname = type(inst).__name__
                if si is not None and len(si.on_wait) > 1 and tname not in exempt:
                    for k, w in enumerate(si.on_wait[:-1]):
                        out.append(mybir.InstEventSemaphore(
                            name=f"{inst.name}-wsplit{k}",
                            engine=inst.engine,
                            ins=[], outs=[],
                            sync_info=mybir.SyncInfo(
                                on_wait=[w], on_update=[]),
                        ))
                    inst.sync_info = mybir.SyncInfo(
                        on_wait=[si.on_wait[-1]],
                        on_update=list(si.on_update),
                    )
                    changed = True
                out.append(inst)
            if changed:
                bb.instructions = out


def _get_nc():
    if "nc" not in _CACHE:
        _CACHE["nc"] = _build_bass()
    return _CACHE["nc"]


# ---------------------------------------------------------------------------
# host orchestration
# ---------------------------------------------------------------------------

def _prep_in_maps(inputs):
    x = np.asarray(inputs["x"], np.float32)
    cos, sin = _freqs(T)  # [T, 32] each
    c32 = np.ascontiguousarray(cos.T)  # [32, T]
    s32 = np.ascontiguousarray(sin.T)

    qnw = np.asarray(inputs["q_norm_w"], np.float32)
    kvw = np.asarray(inputs["kv_norm_w"], np.float32)
    Wqn_r = (np.asarray(inputs["Wq_nope"], np.float32)
             * qnw[:, None]).reshape(96, N_HEAD, NOPE)
    Wqr_r = (np.asarray(inputs["Wq_rope"], np.float32)
             * qnw[:, None]).reshape(96, N_HEAD, ROPE)
    Wkn_r = (np.asarray(inputs["Wk_nope"], np.float32)
             * kvw[:, None]).reshape(32, N_HEAD, NOPE)
    Wv_r = (np.asarray(inputs["Wv"], np.float32)
            * kvw[:, None]).reshape(32, N_HEAD, VDIM)
    Wsk_r = np.asarray(inputs["Wsel_k"], np.float32).reshape(C, N_HEAD, HD)
    Wsv_r = np.asarray(inputs["Wsel_v"], np.float32).reshape(C, N_HEAD, VDIM)
    Wwk_r = np.asarray(inputs["Wwin_k"], np.float32).reshape(C, N_HEAD, HD)
    Wwv_r = np.asarray(inputs["Wwin_v"], np.float32).reshape(C, N_HEAD, VDIM)
    Wkr = np.asarray(inputs["Wk_rope"], np.float32) / N_HEAD  # [C, 64]
    Wcq = np.asarray(inputs["Wcq"], np.float32)
    Wckv = np.asarray(inputs["Wckv"], np.float32)
    Wp = np.asarray(inputs["Wproj"], np.float32)

    per_batch = []
    for b in range(B):
        xb = x[b]
        glog = (xb @ np.asarray(inputs["Wgate"], np.float32)).mean(0)
        g = np.exp(glog - glog.max())
        g = (g / g.sum()).astype(np.float32)
        scores = (xb @ np.asarray(inputs["W_imp"], np.float32))[:, 0]
        idx = np.sort(np.argpartition(-scores, KEEP - 1)[:KEEP])
        per_batch.append((xb, xb[idx], g))

    in_maps = []
    for core in range(N_CORES):
        b, hg = divmod(core, N_CORES // B)
        xb, sel, g = per_batch[b]
        hsl = slice(hg * HPC, hg * HPC + HPC)

        blob = np.zeros((128, NB), np.float32)

        def put(col, w):  # w [C, X] -> 2 cin chunks side by side
            Xw = w.shape[1]
            for cc in range(2):
                blob[:, col + cc * Xw : col + (cc + 1) * Xw] = \
                    w[cc * 128 : (cc + 1) * 128]

        put(XT, xb.T)
        put(SEL, sel.T)
        put(WSKN, Wsk_r[:, hsl, :NOPE].reshape(C, -1))
        put(WSKR, Wsk_r[:, hsl, NOPE:].reshape(C, -1))
        put(WSV, (Wsv_r[:, hsl] * g[1]).reshape(C, -1))
        put(WWKN, Wwk_r[:, hsl, :NOPE].reshape(C, -1))
        put(WWKR, Wwk_r[:, hsl, NOPE:].reshape(C, -1))
        put(WWV, (Wwv_r[:, hsl] * g[2]).reshape(C, -1))
        put(WCQ, Wcq)
        put(WCKV, Wckv)
        put(WKR, Wkr)

        b96 = np.concatenate(
            [Wqn_r[:, hsl].reshape(96, -1), Wqr_r[:, hsl].reshape(96, -1)], 1)
        b32 = np.concatenate(
            [c32, s32, Wkn_r[:, hsl].reshape(32, -1),
             (Wv_r[:, hsl] * g[0]).reshape(32, -1)], 1)

        in_maps.append({
            "blob": blob.astype(BF),
            "b96": np.ascontiguousarray(b96).astype(BF),
            "b32": np.ascontiguousarray(b32).astype(BF),
        })
    return in_maps, Wp


def _run(inputs, trace=False):
    from concourse.bass_utils import run_bass_kernel_spmd

    nc = _get_nc()
    in_maps, Wp = _prep_in_maps(inputs)
    res = run_bass_kernel_spmd(nc, in_maps, list(range(N_CORES)), trace=trace)
    out = np.zeros((B, T, C), np.float32)
    for core in range(N_CORES):
        b, hg = divmod(core, N_CORES // B)
        acc = np.asarray(res.results[core]["outT"], np.float32)  # [128, T]
        out[b] += acc.T @ Wp[hg * 128 : hg * 128 + 128]
    return out, res


def kernel(**inputs):
    out, _ = _run(inputs, trace=False)
    return out


# revision 24
# speedup vs baseline: 3.4270x; 1.2091x over previous
"""nn_Attn_9715216024104 — sparse attention (MLA + top-k select + sliding window).

Sharding: 8 cores = 2 batches x 4 head-groups (4 heads each). Each core runs
one Bass/Tile kernel computing its 4 heads' three attention branches
(S^T layout, exp softmax without max-subtraction — scores are <0.5 — with
ones-column-folded Z rows in the PV matmul).

This revision minimizes per-call host<->device traffic (the axon tunnel runs
at ~45 MB/s, so bytes dominate the wall clock):
- Inputs are packed into 3 DRAM params (blob[128,NB], b96, b32) instead of 20.
- cq/ckv RMS-norm and the shared roped kr are computed ON DEVICE from x
  (norm weights folded into downstream projections on the host; rms factor
  via ones-matmul column reduce + Sqrt activation + DVE reciprocal + rank-1
  f32 matmul partition-broadcast).
- Rope cos/sin tables ship compact as [32,T] and are replicated/sign-folded
  on device; "swapped" rope projection weights are built on device by
  column-half swaps instead of being shipped twice.
- The epilogue (divide by Z, gate, sum the 3 branches) runs ON DEVICE:
  branch gates are folded into the V projection weights on the host, evicts
  normalize by the PSUM Z row (DVE reciprocal + rank-1 broadcast) and
  accumulate into a f32 [128,T] tile; output is a single bf16 [128,T] per
  core. Host only applies Wproj per head-group and sums.
- kernel.py also enables JAX's persistent compilation cache so repeated
  run_bass_kernel_spmd calls skip the per-call XLA/NEFF rebuild.

Device layout notes:
- All matmul operands bf16 (rank-1 Z/rms broadcasts use f32); PSUM f32.
- Attention uses S^T tiles [k=128, q] so P^T feeds the PV matmul directly;
  V tiles carry a ones column so the PV matmul also produces Z rows.
- Rope is applied via duplicated "swapped" projection weights:
  rope(x) = x * cos + swap(x) * sgn*sin, with swap folded into a second
  matmul, so DVE only does 2 muls + 1 add.
- Causal / sliding-window masking is done on GPSIMD (affine_select zeroing
  of P^T after exp), keeping TensorE/ACT free of mask work.
"""

import math

import numpy as np
import ml_dtypes

try:  # persistent XLA compilation cache: the per-call jit is a fresh closure
    import jax

    jax.config.update("jax_compilation_cache_dir", "/tmp/jaxcache")
    jax.config.update("jax_persistent_cache_min_compile_time_secs", 0.0)
    jax.config.update("jax_persistent_cache_min_entry_size_bytes", 0)
except Exception:
    pass

BF = ml_dtypes.bfloat16

N_HEAD = 16
NOPE = 32
ROPE = 64
VDIM = 32
HD = NOPE + ROPE  # 96
WINDOW = 128
KEEP = 512
EPS = 1e-6
N_CORES = 4
HPC = 4  # heads per quad (the inner program unit)
QUADS = 2  # head-quads per core -> 8 heads per core
B, T, C = 2, 2048, 256
QT = 512  # q tile (free dim)
NJQ = T // QT  # 4 q tiles
NKB = T // 128  # 16 k blocks
SCALE = 1.0 / math.sqrt(HD)

# blob column offsets (all bf16, [128, NB]); per-quad weight sections are
# QW columns apart
XT = 0  # x^T, 2 cin chunks       [128, 2*T]
SEL = XT + 2 * T  # sel^T, 2 cin chunks     [128, 2*KEEP]
WSKN = SEL + 2 * KEEP  # Wsel_k nope, 2 chunks   [128, 2*128]
WSKR = WSKN + 256  # Wsel_k rope, 2 chunks   [128, 2*256]
WSV = WSKR + 512  # Wsel_v (gated), 2 chunks [128, 2*128]
WWKN = WSV + 256  # Wwin_k nope             [128, 2*128]
WWKR = WWKN + 256  # Wwin_k rope             [128, 2*256]
WWV = WWKR + 512  # Wwin_v (gated)          [128, 2*128]
QW = 2048  # per-quad stride of the WSKN..WWV block
WCQ = WSKN + QUADS * QW  # Wcq              [128, 2*96]
WCKV = WCQ + 192  # Wckv                    [128, 2*32]
WKR = WCKV + 64  # Wk_rope/N_HEAD          [128, 2*64]
NB = WKR + 128

# b96 columns ([96, N96]); per-quad stride QW96
WQN = 0  # Wq_nope (norm-folded)  [96, 128]
WQR = 128  # Wq_rope (norm-folded)  [96, 256]
QW96 = 384
N96 = QUADS * QW96

# b32 columns ([32, N32]); per-quad stride QW32 for the weight tail
C32 = 0  # cos^T [32, T]
S32 = T  # sin^T [32, T]
WKN = 2 * T  # Wk_nope (norm-folded) [32, 128]
WV = 2 * T + 128  # Wv (norm- and gate-folded) [32, 128]
QW32 = 256
N32 = 2 * T + QUADS * QW32

_CACHE = {}


# ---------------------------------------------------------------------------
# host-side helpers
# ---------------------------------------------------------------------------

def _freqs(t):
    f = 1.0 / 1e4 ** (np.arange(0, ROPE, 2, dtype=np.float32) / ROPE)
    ang = np.outer(np.arange(t, dtype=np.float32), f)
    return np.cos(ang).astype(np.float32), np.sin(ang).astype(np.float32)


# ---------------------------------------------------------------------------
# bass program (built once; identical for all 8 cores)
# ---------------------------------------------------------------------------

def _build_bass(legalize=True):
    import concourse.bass as bass
    import concourse.mybir as mybir
    import concourse.tile as tile

    f32 = mybir.dt.float32
    bf16 = mybir.dt.bfloat16
    EXP = mybir.ActivationFunctionType.Exp
    SQRT = mybir.ActivationFunctionType.Sqrt
    GE = mybir.AluOpType.is_ge

    nc = bass.Bass(target_bir_lowering=False, debug=False)

    d_blob = nc.declare_dram_parameter("blob", [128, NB], bf16, isOutput=False)
    d_b96 = nc.declare_dram_parameter("b96", [96, N96], bf16, isOutput=False)
    d_b32 = nc.declare_dram_parameter("b32", [32, N32], bf16, isOutput=False)
    d_out = nc.declare_dram_parameter("outT", [QUADS, 128, T], bf16,
                                      isOutput=True)

    def asl(base, s):  # absolute blob column slice
        return slice(base + s.start, base + s.stop)

    with tile.TileContext(nc) as tc:
        with (
            tc.tile_pool(name="const", bufs=1) as cpool,
            tc.tile_pool(name="big", bufs=1) as bpool,
            tc.tile_pool(name="pt", bufs=3) as ptpool,
            tc.tile_pool(name="sc", bufs=4) as scpool,
        ):
            _dma_engines = [nc.sync, nc.gpsimd, nc.scalar]
            _dma_rr = [0]

            def _dma(out, in_):
                eng = _dma_engines[_dma_rr[0] % len(_dma_engines)]
                _dma_rr[0] += 1
                eng.dma_start(out=out, in_=in_)

            s_blob = cpool.tile([128, NB], bf16, name="blob", tag="blob")
            s_b96 = cpool.tile([96, N96], bf16, name="b96", tag="b96")
            s_b32 = cpool.tile([32, N32], bf16, name="b32", tag="b32")

            # small/early-needed first; big x/sel tensors split for overlap
            _dma(s_b32[:, :], d_b32[:, :])
            _dma(s_b96[:, :], d_b96[:, :])
            _dma(s_blob[:, WSKN:NB], d_blob[:, WSKN:NB])
            _dma(s_blob[:, XT : XT + T], d_blob[:, XT : XT + T])
            _dma(s_blob[:, XT + T : XT + 2 * T], d_blob[:, XT + T : XT + 2 * T])
            _dma(s_blob[:, SEL : SEL + 2 * KEEP], d_blob[:, SEL : SEL + 2 * KEEP])

            def xT_ap(cc, ts_):
                return s_blob[:, asl(XT + cc * T, ts_)]

            # ---- rope tables [128, T] from compact [32, T] + sign folding ----
            c128 = cpool.tile([128, T], bf16, name="c128", tag="c128")
            s128 = cpool.tile([128, T], bf16, name="s128", tag="s128")
            for blk in range(4):
                rs = slice(32 * blk, 32 * blk + 32)
                nc.scalar.copy(c128[rs, :], s_b32[:, C32 : C32 + T])
                if blk % 2 == 0:
                    nc.vector.tensor_scalar_mul(
                        s128[rs, :], s_b32[:, S32 : S32 + T], -1.0)
                else:
                    nc.gpsimd.tensor_copy(s128[rs, :], s_b32[:, S32 : S32 + T])

            # ---- swapped rope weights built on device (column-half swap) ----
            wqrS = [cpool.tile([96, 256], bf16, name=f"wqrS{q}",
                               tag=f"wqrS{q}") for q in range(QUADS)]
            wskrS = [cpool.tile([128, 2, 256], bf16, name=f"wskrS{q}",
                                tag=f"wskrS{q}") for q in range(QUADS)]
            wwkrS = [cpool.tile([128, 2, 256], bf16, name=f"wwkrS{q}",
                                tag=f"wwkrS{q}") for q in range(QUADS)]
            wkrS = cpool.tile([128, 2, 64], bf16, tag="wkrS")
            _sw_rr = [0]

            def _sweng():
                eng = (nc.vector, nc.gpsimd, nc.scalar)[_sw_rr[0] % 3]
                _sw_rr[0] += 1
                return eng

            def swap_into(dst_ap, src_ap, nh):
                # both viewed [P, nh, 2, 32]; swap axis-2 halves
                dv = dst_ap.rearrange("p (h two c) -> p h two c", two=2, c=32)
                sv = src_ap.rearrange("p (h two c) -> p h two c", two=2, c=32)
                for half in range(2):
                    eng = _sweng()
                    (eng.tensor_copy if eng is not nc.scalar else eng.copy)(
                        dv[:, :, half, :], sv[:, :, 1 - half, :])

            for hq in range(QUADS):
                swap_into(wqrS[hq][:, :],
                          s_b96[:, hq * QW96 + WQR : hq * QW96 + WQR + 256], 4)
                for cc in range(2):
                    swap_into(
                        wskrS[hq][:, cc, :],
                        s_blob[:, hq * QW + WSKR + cc * 256 :
                               hq * QW + WSKR + cc * 256 + 256], 4)
                    swap_into(
                        wwkrS[hq][:, cc, :],
                        s_blob[:, hq * QW + WWKR + cc * 256 :
                               hq * QW + WWKR + cc * 256 + 256], 4)
            for cc in range(2):
                swap_into(wkrS[:, cc, :],
                          s_blob[:, WKR + cc * 64 : WKR + cc * 64 + 64], 1)

            # ---- constants for reductions/broadcasts ----
            ones96c = cpool.tile([96, 1], bf16, tag="o96c")
            ones32c = cpool.tile([32, 1], bf16, tag="o32c")
            ones96r = cpool.tile([1, 96], f32, tag="o96r")
            ones32r = cpool.tile([1, 32], f32, tag="o32r")
            epsc = cpool.tile([1, 1], f32, tag="epsc")
            nc.vector.memset(ones96c[:, :], 1.0)
            nc.vector.memset(ones32c[:, :], 1.0)
            nc.vector.memset(ones96r[:, :], 1.0)
            nc.vector.memset(ones32r[:, :], 1.0)
            nc.vector.memset(epsc[:, :], EPS)

            # ---- assembled per-head [96, h, T] q/k layouts ----
            cqT = bpool.tile([96, T], bf16)   # rms-normalized cq^T
            ckvT = bpool.tile([32, T], bf16)  # rms-normalized ckv^T
            q96 = bpool.tile([96, 4, T], bf16)
            k96 = bpool.tile([96, 4, T], bf16)     # branch 1 (kn | shared kr)
            ks96 = bpool.tile([96, 4, KEEP], bf16)  # branch 2
            kw96 = bpool.tile([96, 4, T], bf16)    # branch 3
            v1 = bpool.tile([128, NKB, 132], bf16)
            vs = bpool.tile([128, 4, 132], bf16)
            vw = bpool.tile([128, NKB, 132], bf16)
            acc = bpool.tile([128, T], f32)        # gated, normalized output
            outb = bpool.tile([128, T], bf16)

            with (
                tc.tile_pool(name="pp", bufs=2, space=bass.MemorySpace.PSUM) as pp,
                tc.tile_pool(name="sgp", bufs=2, space=bass.MemorySpace.PSUM) as sgp,
                tc.tile_pool(name="otp", bufs=2, space=bass.MemorySpace.PSUM) as otp,
            ):
                def rmsnorm_proj(dst, p, wcol, wwid, ones_col, ones_row, inv_n):
                    """dst[p, T] <- rms-normalized W^T x^T (tokens on free dim).
                    rms factor: ones-matmul column sum of squares -> Sqrt ACT
                    -> DVE reciprocal -> rank-1 f32 matmul broadcast."""
                    for t4 in range(NJQ):
                        ts_ = slice(t4 * QT, t4 * QT + QT)
                        ps = pp.tile([128, QT], f32, tag="p1",
                                     padded_shape=[128, QT])
                        for cc in range(2):
                            nc.tensor.matmul(
                                ps[0:p, :],
                                s_blob[:, wcol + cc * wwid : wcol + (cc + 1) * wwid],
                                xT_ap(cc, ts_), start=(cc == 0), stop=(cc == 1),
                            )
                        pre = scpool.tile([p, QT], bf16, tag="pre")
                        nc.scalar.copy(pre[:, :], ps[0:p, :])
                        sq = scpool.tile([p, QT], bf16, tag="sq")
                        nc.vector.tensor_mul(sq[:, :], pre[:, :], pre[:, :])
                        ps2 = pp.tile([1, QT], f32, tag="p1",
                                      padded_shape=[128, QT])
                        nc.tensor.matmul(ps2[:, :], ones_col[:, :], sq[:, :],
                                         start=True, stop=True)
                        srow = scpool.tile([1, QT], f32, tag="srow", bufs=2)
                        nc.scalar.activation(srow[:, :], ps2[:, :], SQRT,
                                             bias=epsc[:, :], scale=inv_n)
                        rin = scpool.tile([1, QT], f32, tag="rin", bufs=2)
                        nc.vector.reciprocal(rin[:, :], srow[:, :])
                        bc = pp.tile([128, QT], f32, tag="p1",
                                     padded_shape=[128, QT])
                        nc.tensor.matmul(bc[0:p, :], ones_row[:, 0:p],
                                         rin[:, :], start=True, stop=True)
                        nc.vector.tensor_mul(dst[:, ts_], pre[:, :], bc[0:p, :])

                def kr_build():
                    """k96[0:64, h, :] <- rope((x @ Wk_rope)/N_HEAD), all heads."""
                    for t4 in range(NJQ):
                        ts_ = slice(t4 * QT, t4 * QT + QT)
                        pr = pp.tile([128, QT], f32, tag="p1",
                                     padded_shape=[128, QT])
                        psw = pp.tile([128, QT], f32, tag="p1",
                                      padded_shape=[128, QT])
                        for cc in range(2):
                            nc.tensor.matmul(
                                pr[0:64, :],
                                s_blob[:, WKR + cc * 64 : WKR + cc * 64 + 64],
                                xT_ap(cc, ts_), start=(cc == 0), stop=(cc == 1),
                            )
                        for cc in range(2):
                            nc.tensor.matmul(
                                psw[0:64, :], wkrS[:, cc, :], xT_ap(cc, ts_),
                                start=(cc == 0), stop=(cc == 1),
                            )
                        t1 = scpool.tile([64, QT], bf16, tag="rt1")
                        t2 = scpool.tile([64, QT], bf16, tag="rt2")
                        nc.vector.tensor_mul(t1[:, :], pr[0:64, :],
                                             c128[0:64, ts_])
                        nc.vector.tensor_mul(t2[:, :], psw[0:64, :],
                                             s128[0:64, ts_])
                        nc.gpsimd.tensor_add(k96[0:64, 0, ts_], t1[:, :],
                                             t2[:, :])
                        for h in range(1, 4):
                            nc.scalar.copy(k96[0:64, h, ts_], k96[0:64, 0, ts_])

                def proj_nope(dest96, lhsW, rhs_of, tlen, nacc, eng=None):
                    """4-head nope projection, split per head into
                    dest96[64:96, h, ts]."""
                    step = min(tlen, QT)
                    for t4 in range(max(1, tlen // step)):
                        ts_ = slice(t4 * step, t4 * step + step)
                        ps = pp.tile([128, step], f32, tag="p1",
                                     padded_shape=[128, QT])
                        for cc in range(nacc):
                            nc.tensor.matmul(
                                ps[:], lhsW(cc), rhs_of(cc, ts_),
                                start=(cc == 0), stop=(cc == nacc - 1),
                            )
                        for h in range(4):
                            if eng == "act":
                                nc.scalar.copy(
                                    dest96[64:96, h, ts_],
                                    ps[32 * h : 32 * h + 32, :],
                                )
                            else:
                                nc.vector.tensor_copy(
                                    dest96[64:96, h, ts_],
                                    ps[32 * h : 32 * h + 32, :],
                                )

                def rope_proj(dest96, hpair, cos_sl, lhs_raw, lhs_sw,
                              rhs_list, tlen, ts_):
                    """Rope for one head-pair chunk; writes per-head rows
                    dest96[0:64, h, ts]."""
                    pr = pp.tile([128, tlen], f32, tag="p1",
                                 padded_shape=[128, QT])
                    psw = pp.tile([128, tlen], f32, tag="p1",
                                  padded_shape=[128, QT])
                    ncc = len(rhs_list)
                    for cc, rhs in enumerate(rhs_list):
                        nc.tensor.matmul(
                            pr[:], lhs_raw[cc], rhs,
                            start=(cc == 0), stop=(cc == ncc - 1),
                        )
                    for cc, rhs in enumerate(rhs_list):
                        nc.tensor.matmul(
                            psw[:], lhs_sw[cc], rhs,
                            start=(cc == 0), stop=(cc == ncc - 1),
                        )
                    t1 = scpool.tile([128, tlen], bf16, tag="rt1")
                    t2 = scpool.tile([128, tlen], bf16, tag="rt2")
                    nc.vector.tensor_mul(t1[:], pr[:], c128[:, cos_sl])
                    nc.vector.tensor_mul(t2[:], psw[:], s128[:, cos_sl])
                    for hi in range(2):
                        h = 2 * hpair + hi
                        hr = slice(64 * hi, 64 * hi + 64)
                        nc.gpsimd.tensor_add(
                            dest96[0:64, h, ts_], t1[hr, :], t2[hr, :]
                        )

                def rope_all(dest96, lhsW, lhsWS, rhs_of, tlen, nacc):
                    for j in range(2):
                        hs = slice(j * 128, j * 128 + 128)
                        step = min(tlen, QT)
                        for t4 in range(max(1, tlen // step)):
                            ts_ = slice(t4 * step, t4 * step + step)
                            rope_proj(
                                dest96, j, ts_,
                                [lhsW(cc, hs) for cc in range(nacc)],
                                [lhsWS(cc, hs) for cc in range(nacc)],
                                [rhs_of(cc, ts_) for cc in range(nacc)],
                                step, ts_,
                            )

                def v_tile(dest, nblk, lhs_fn, rhs_fn, nacc):
                    nc.vector.memset(dest[:, :, slice(32, 132, 33)], 1.0)
                    for tb in range(nblk):
                        ps = pp.tile([128, 128], f32, tag="p1",
                                     padded_shape=[128, QT])
                        for cc in range(nacc):
                            nc.tensor.matmul(
                                ps[:], lhs_fn(cc, tb), rhs_fn(cc),
                                start=(cc == 0), stop=(cc == nacc - 1),
                            )
                        nc.vector.tensor_copy(
                            dest[:, tb, :].rearrange(
                                "p (h c) -> p h c", h=4)[:, :, 0:32],
                            ps[:].rearrange("p (h c) -> p h c", h=4),
                        )

                def evict(jq, br, hp, ots):
                    """Normalize by the PSUM Z row and accumulate (gates are
                    folded into the V weights host-side)."""
                    oc = slice(jq * QT, jq * QT + QT)
                    for hi in range(2):
                        h = 2 * hp + hi
                        zi = scpool.tile([1, QT], f32, tag="zi", bufs=2)
                        nc.vector.reciprocal(zi[:, :], ots[hi][32:33, :])
                        zb = pp.tile([32, QT], f32, tag="p1",
                                     padded_shape=[128, QT])
                        nc.tensor.matmul(zb[:, :], ones32r[:, :], zi[:, :],
                                         start=True, stop=True)
                        # two PSUM srcs in one DVE op are illegal; stage zb
                        zbs = scpool.tile([32, QT], f32, tag="zbs", bufs=2)
                        nc.vector.tensor_copy(zbs[:, :], zb[:, :])
                        hr = slice(32 * h, 32 * h + 32)
                        ar = acc[hr, oc]
                        if br == 2:  # first writer of this acc region
                            nc.vector.tensor_mul(ar, ots[hi][0:32, :],
                                                 zbs[:, :])
                        else:
                            # tm band matches acc's partitions: walrus wants
                            # TT *inputs* on the same start partition
                            tm = scpool.tile([128, QT], f32, tag="tm", bufs=2)
                            nc.vector.tensor_mul(tm[hr, :], ots[hi][0:32, :],
                                                 zbs[:, :])
                            nc.gpsimd.tensor_add(ar, ar, tm[hr, :])

                def branch12(br, jqs):
                    kT = k96 if br == 1 else ks96
                    vt = v1 if br == 1 else vs
                    for jq in jqs:
                        nkb = 4 * (jq + 1) if br == 1 else 4
                        for hp in range(2):
                            ots = [
                                otp.tile([33, QT], f32, name=f"ot{i}",
                                         tag=f"ot{i}", bufs=1)
                                for i in range(2)
                            ]
                            for kb in range(nkb):
                                ksl = slice(kb * 128, kb * 128 + 128)
                                sg = sgp.tile([128, 1024], f32, tag="sg")
                                # on diagonal blocks only the causally-valid
                                # q columns [128i, QT) are ever consumed
                                off = (128 * (kb - 4 * jq)
                                       if br == 1 and kb >= 4 * jq else 0)
                                for hi in range(2):
                                    h = 2 * hp + hi
                                    nc.tensor.matmul(
                                        sg[:, hi * QT + off : hi * QT + QT],
                                        kT[:, h, ksl],
                                        q96[:, h,
                                            jq * QT + off : jq * QT + QT],
                                        start=True, stop=True,
                                    )
                                pt = ptpool.tile([128, 1024], bf16, tag="pt")
                                diag = br == 1 and kb >= 4 * jq
                                if diag:
                                    # exp only the causally-valid columns;
                                    # zero the rest, then mask the triangle
                                    i = kb - 4 * jq
                                    vq = slice(128 * i, QT)
                                    sgv = sg[:].rearrange(
                                        "p (h q) -> p h q", h=2)
                                    ptv = pt[:].rearrange(
                                        "p (h q) -> p h q", h=2)
                                    if i > 0:
                                        nc.gpsimd.memset(
                                            ptv[:, :, 0 : 128 * i], 0.0)
                                    nc.scalar.activation(
                                        ptv[:, :, vq], sgv[:, :, vq],
                                        EXP, scale=SCALE,
                                    )
                                    nc.gpsimd.affine_select(
                                        out=ptv[:, :, vq], in_=ptv[:, :, vq],
                                        compare_op=GE, fill=0.0,
                                        base=0,
                                        pattern=[[0, 2], [1, QT - 128 * i]],
                                        channel_multiplier=-1,
                                    )
                                else:
                                    nc.scalar.activation(
                                        pt[:], sg[:], EXP, scale=SCALE)
                                for hi in range(2):
                                    h = 2 * hp + hi
                                    nc.tensor.matmul(
                                        ots[hi][:],
                                        vt[:, kb, 33 * h : 33 * h + 33],
                                        pt[:, hi * QT : hi * QT + QT],
                                        start=(kb == 0), stop=(kb == nkb - 1),
                                    )
                            evict(jq, br, hp, ots)

                def branch3(jqs):
                    for jq in jqs:
                        for hp in range(2):
                            ots = [
                                otp.tile([33, QT], f32, name=f"ot{i}",
                                         tag=f"ot{i}", bufs=1)
                                for i in range(2)
                            ]
                            for qcp in range(2):  # pairs of 128-q chunks
                                sg = sgp.tile([128, 1024], f32, tag="sg")
                                for qcs in range(2):
                                    qb = 4 * jq + 2 * qcp + qcs
                                    qbs = slice(qb * 128, qb * 128 + 128)
                                    for hi in range(2):
                                        h = 2 * hp + hi
                                        for ki, kb in enumerate((qb - 1, qb)):
                                            col = slice(
                                                qcs * 512 + hi * 256 + ki * 128,
                                                qcs * 512 + hi * 256 + ki * 128
                                                + 128)
                                            if kb < 0:
                                                nc.vector.memset(
                                                    sg[:, col], 0.0)
                                                continue
                                            ksl = slice(kb * 128,
                                                        kb * 128 + 128)
                                            nc.tensor.matmul(
                                                sg[:, col], kw96[:, h, ksl],
                                                q96[:, h, qbs],
                                                start=True, stop=True,
                                            )
                                pt = ptpool.tile([128, 1024], bf16, tag="pt")
                                nc.scalar.activation(pt[:], sg[:], EXP,
                                                     scale=SCALE)
                                ptv = pt[:].rearrange("p (c q) -> p c q", c=8)
                                nc.gpsimd.affine_select(  # diag: col >= row
                                    out=ptv[:, slice(1, 8, 2)],
                                    in_=ptv[:, slice(1, 8, 2)],
                                    compare_op=GE, fill=0.0, base=0,
                                    pattern=[[0, 4], [1, 128]],
                                    channel_multiplier=-1,
                                )
                                nc.gpsimd.affine_select(  # prev: row > col
                                    out=ptv[:, slice(0, 8, 2)],
                                    in_=ptv[:, slice(0, 8, 2)],
                                    compare_op=GE, fill=0.0, base=-1,
                                    pattern=[[0, 4], [-1, 128]],
                                    channel_multiplier=1,
                                )
                                for qcs in range(2):
                                    qb = 4 * jq + 2 * qcp + qcs
                                    for hi in range(2):
                                        h = 2 * hp + hi
                                        for ki, kb in enumerate((qb - 1, qb)):
                                            if kb < 0:
                                                continue
                                            col = slice(
                                                qcs * 512 + hi * 256 + ki * 128,
                                                qcs * 512 + hi * 256 + ki * 128
                                                + 128)
                                            oc = slice(
                                                (2 * qcp + qcs) * 128,
                                                (2 * qcp + qcs) * 128 + 128)
                                            nc.tensor.matmul(
                                                ots[hi][:, oc],
                                                vw[:, kb,
                                                   33 * h : 33 * h + 33],
                                                pt[:, col],
                                                start=(kb == max(qb - 1, 0)),
                                                stop=(kb == qb),
                                            )
                            evict(jq, 3, hp, ots)

                # ---- emission: per-quad, q+b2 deps first (b2 is ACT-dense
                # and can start while the rest of the assembly runs on DVE).
                # Shared prep (cq/ckv/kr/tables) runs once; per-quad tiles
                # (q96, k96 nope rows, kw96, v*, acc) are rebuilt each quad.
                rmsnorm_proj(cqT, 96, WCQ, 96, ones96c, ones96r, 1.0 / 96)
                for hq in range(QUADS):
                    bo = hq * QW       # blob per-quad weight offset
                    b9 = hq * QW96     # b96 per-quad offset
                    b3 = hq * QW32     # b32 per-quad offset
                    proj_nope(q96,
                              lambda cc: s_b96[:, b9 + WQN : b9 + WQN + 128],
                              lambda cc, ts_: cqT[:, ts_], T, 1)
                    rope_all(q96,
                             lambda cc, hs: s_b96[:, asl(b9 + WQR, hs)],
                             lambda cc, hs: wqrS[hq][:, hs],
                             lambda cc, ts_: cqT[:, ts_], T, 1)
                    proj_nope(ks96,
                              lambda cc: s_blob[:, bo + WSKN + cc * 128 :
                                                bo + WSKN + cc * 128 + 128],
                              lambda cc, ts_: s_blob[:, asl(SEL + cc * KEEP, ts_)],
                              KEEP, 2)
                    rope_all(ks96,
                             lambda cc, hs: s_blob[:, asl(bo + WSKR + cc * 256, hs)],
                             lambda cc, hs: wskrS[hq][:, cc, hs],
                             lambda cc, ts_: s_blob[:, asl(SEL + cc * KEEP, ts_)],
                             KEEP, 2)
                    v_tile(
                        vs, 4,
                        lambda cc, tb: s_blob[:, SEL + cc * KEEP + tb * 128 :
                                              SEL + cc * KEEP + tb * 128 + 128],
                        lambda cc: s_blob[:, bo + WSV + cc * 128 :
                                          bo + WSV + cc * 128 + 128],
                        2,
                    )
                    branch12(2, range(NJQ))
                    if hq == 0:
                        # shared kv-path prep overlaps with branch 2
                        rmsnorm_proj(ckvT, 32, WCKV, 32, ones32c, ones32r,
                                     1.0 / 32)
                        kr_build()
                    proj_nope(k96, lambda cc: s_b32[:, b3 + WKN : b3 + WKN + 128],
                              lambda cc, ts_: ckvT[:, ts_], T, 1)
                    v_tile(
                        v1, NKB,
                        lambda cc, tb: ckvT[:, tb * 128 : tb * 128 + 128],
                        lambda cc: s_b32[:, b3 + WV : b3 + WV + 128], 1,
                    )
                    branch12(1, range(NJQ - 1))
                    proj_nope(kw96,
                              lambda cc: s_blob[:, bo + WWKN + cc * 128 :
                                                bo + WWKN + cc * 128 + 128],
                              lambda cc, ts_: xT_ap(cc, ts_), T, 2)
                    rope_all(kw96,
                             lambda cc, hs: s_blob[:, asl(bo + WWKR + cc * 256, hs)],
                             lambda cc, hs: wwkrS[hq][:, cc, hs],
                             lambda cc, ts_: xT_ap(cc, ts_), T, 2)
                    v_tile(
                        vw, NKB,
                        lambda cc, tb: xT_ap(cc, slice(tb * 128, tb * 128 + 128)),
                        lambda cc: s_blob[:, bo + WWV + cc * 128 :
                                          bo + WWV + cc * 128 + 128],
                        2,
                    )
                    branch3(range(NJQ))
                    branch12(1, [NJQ - 1])

                    # bf16 cast + output DMA for this quad (split by jq)
                    for jq in range(NJQ):
                        oc = slice(jq * QT, jq * QT + QT)
                        nc.scalar.copy(outb[:, oc], acc[:, oc])
                        _dma(d_out[hq, :, oc], outb[:, oc])

    if legalize:
        _legalize_pe_waits(nc, mybir)
    return nc


def _legalize_pe_waits(nc, mybir):
    """This walrus build encodes at most ONE sync-wait per compute
    instruction, but Tile emits up to 3. Split excess waits into standalone
    same-engine InstEventSemaphore waits placed immediately before the
    instruction (program point unchanged, so no deadlock risk)."""
    exempt = ("InstEventSemaphore", "InstNoOp",
              "InstUnconditionalBranch", "InstCall", "InstISA")
    for f in nc.m.functions:
        for bb in f.blocks:
            out = []
            changed = False
            for inst in bb.instructions:
                si = inst.sync_info
                tname = type(inst).__name__
                if si is not None and len(si.on_wait) > 1 and tname not in exempt:
                    for k, w in enumerate(si.on_wait[:-1]):
                        out.append(mybir.InstEventSemaphore(
                            name=f"{inst.name}-wsplit{k}",
                            engine=inst.engine,
                            ins=[], outs=[],
                            sync_info=mybir.SyncInfo(
                                on_wait=[w], on_update=[]),
                        ))
                    inst.sync_info = mybir.SyncInfo(
                        on_wait=[si.on_wait[-1]],
                        on_update=list(si.on_update),
                    )
                    changed = True
                out.append(inst)
            if changed:
                bb.instructions = out


def _get_nc():
    if "nc" not in _CACHE:
        _CACHE["nc"] = _build_bass()
    return _CACHE["nc"]


# ---------------------------------------------------------------------------
# host orchestration
# ---------------------------------------------------------------------------

def _prep_in_maps(inputs):
    x = np.asarray(inputs["x"], np.float32)
    cos, sin = _freqs(T)  # [T, 32] each
    c32 = np.ascontiguousarray(cos.T)  # [32, T]
    s32 = np.ascontiguousarray(sin.T)

    qnw = np.asarray(inputs["q_norm_w"], np.float32)
    kvw = np.asarray(inputs["kv_norm_w"], np.float32)
    Wqn_r = (np.asarray(inputs["Wq_nope"], np.float32)
             * qnw[:, None]).reshape(96, N_HEAD, NOPE)
    Wqr_r = (np.asarray(inputs["Wq_rope"], np.float32)
             * qnw[:, None]).reshape(96, N_HEAD, ROPE)
    Wkn_r = (np.asarray(inputs["Wk_nope"], np.float32)
             * kvw[:, None]).reshape(32, N_HEAD, NOPE)
    Wv_r = (np.asarray(inputs["Wv"], np.float32)
            * kvw[:, None]).reshape(32, N_HEAD, VDIM)
    Wsk_r = np.asarray(inputs["Wsel_k"], np.float32).reshape(C, N_HEAD, HD)
    Wsv_r = np.asarray(inputs["Wsel_v"], np.float32).reshape(C, N_HEAD, VDIM)
    Wwk_r = np.asarray(inputs["Wwin_k"], np.float32).reshape(C, N_HEAD, HD)
    Wwv_r = np.asarray(inputs["Wwin_v"], np.float32).reshape(C, N_HEAD, VDIM)
    Wkr = np.asarray(inputs["Wk_rope"], np.float32) / N_HEAD  # [C, 64]
    Wcq = np.asarray(inputs["Wcq"], np.float32)
    Wckv = np.asarray(inputs["Wckv"], np.float32)
    Wp = np.asarray(inputs["Wproj"], np.float32)

    per_batch = []
    for b in range(B):
        xb = x[b]
        glog = (xb @ np.asarray(inputs["Wgate"], np.float32)).mean(0)
        g = np.exp(glog - glog.max())
        g = (g / g.sum()).astype(np.float32)
        scores = (xb @ np.asarray(inputs["W_imp"], np.float32))[:, 0]
        idx = np.sort(np.argpartition(-scores, KEEP - 1)[:KEEP])
        per_batch.append((xb, xb[idx], g))

    in_maps = []
    for core in range(N_CORES):
        b, hg2 = divmod(core, N_CORES // B)
        xb, sel, g = per_batch[b]

        blob = np.zeros((128, NB), np.float32)

        def put(col, w):  # w [C, X] -> 2 cin chunks side by side
            Xw = w.shape[1]
            for cc in range(2):
                blob[:, col + cc * Xw : col + (cc + 1) * Xw] = \
                    w[cc * 128 : (cc + 1) * 128]

        put(XT, xb.T)
        put(SEL, sel.T)
        b96_parts, b32_parts = [], [c32, s32]
        for hq in range(QUADS):
            gq = hg2 * QUADS + hq  # global head-quad
            hsl = slice(gq * HPC, gq * HPC + HPC)
            bo = hq * QW
            put(bo + WSKN, Wsk_r[:, hsl, :NOPE].reshape(C, -1))
            put(bo + WSKR, Wsk_r[:, hsl, NOPE:].reshape(C, -1))
            put(bo + WSV, (Wsv_r[:, hsl] * g[1]).reshape(C, -1))
            put(bo + WWKN, Wwk_r[:, hsl, :NOPE].reshape(C, -1))
            put(bo + WWKR, Wwk_r[:, hsl, NOPE:].reshape(C, -1))
            put(bo + WWV, (Wwv_r[:, hsl] * g[2]).reshape(C, -1))
            b96_parts += [Wqn_r[:, hsl].reshape(96, -1),
                          Wqr_r[:, hsl].reshape(96, -1)]
            b32_parts += [Wkn_r[:, hsl].reshape(32, -1),
                          (Wv_r[:, hsl] * g[0]).reshape(32, -1)]
        put(WCQ, Wcq)
        put(WCKV, Wckv)
        put(WKR, Wkr)

        b96 = np.concatenate(b96_parts, 1)
        b32 = np.concatenate(b32_parts, 1)

        in_maps.append({
            "blob": blob.astype(BF),
            "b96": np.ascontiguousarray(b96).astype(BF),
            "b32": np.ascontiguousarray(b32).astype(BF),
        })
    return in_maps, Wp


def _run(inputs, trace=False):
    from concourse.bass_utils import run_bass_kernel_spmd

    nc = _get_nc()
    in_maps, Wp = _prep_in_maps(inputs)
    res = run_bass_kernel_spmd(nc, in_maps, list(range(N_CORES)), trace=trace)
    out = np.zeros((B, T, C), np.float32)
    for core in range(N_CORES):
        b, hg2 = divmod(core, N_CORES // B)
        accs = np.asarray(res.results[core]["outT"], np.float32)  # [2,128,T]
        for hq in range(QUADS):
            gq = hg2 * QUADS + hq
            out[b] += accs[hq].T @ Wp[gq * 128 : gq * 128 + 128]
    return out, res


def kernel(**inputs):
    out, _ = _run(inputs, trace=False)
    return out


# revision 25
# speedup vs baseline: 3.5288x; 1.0297x over previous
"""nn_Attn_9715216024104 — sparse attention (MLA + top-k select + sliding window).

Sharding: 8 cores = 2 batches x 4 head-groups (4 heads each). Each core runs
one Bass/Tile kernel computing its 4 heads' three attention branches
(S^T layout, exp softmax without max-subtraction — scores are <0.5 — with
ones-column-folded Z rows in the PV matmul).

This revision minimizes per-call host<->device traffic (the axon tunnel runs
at ~45 MB/s, so bytes dominate the wall clock):
- Inputs are packed into 3 DRAM params (blob[128,NB], b96, b32) instead of 20.
- cq/ckv RMS-norm and the shared roped kr are computed ON DEVICE from x
  (norm weights folded into downstream projections on the host; rms factor
  via ones-matmul column reduce + Sqrt activation + DVE reciprocal + rank-1
  f32 matmul partition-broadcast).
- Rope cos/sin tables ship compact as [32,T] and are replicated/sign-folded
  on device; "swapped" rope projection weights are built on device by
  column-half swaps instead of being shipped twice.
- The epilogue (divide by Z, gate, sum the 3 branches) runs ON DEVICE:
  branch gates are folded into the V projection weights on the host, evicts
  normalize by the PSUM Z row (DVE reciprocal + rank-1 broadcast) and
  accumulate into a f32 [128,T] tile; output is a single bf16 [128,T] per
  core. Host only applies Wproj per head-group and sums.
- kernel.py also enables JAX's persistent compilation cache so repeated
  run_bass_kernel_spmd calls skip the per-call XLA/NEFF rebuild.

Device layout notes:
- All matmul operands bf16 (rank-1 Z/rms broadcasts use f32); PSUM f32.
- Attention uses S^T tiles [k=128, q] so P^T feeds the PV matmul directly;
  V tiles carry a ones column so the PV matmul also produces Z rows.
- Rope is applied via duplicated "swapped" projection weights:
  rope(x) = x * cos + swap(x) * sgn*sin, with swap folded into a second
  matmul, so DVE only does 2 muls + 1 add.
- Causal / sliding-window masking is done on GPSIMD (affine_select zeroing
  of P^T after exp), keeping TensorE/ACT free of mask work.
"""

import math

import numpy as np
import ml_dtypes

try:  # persistent XLA compilation cache: the per-call jit is a fresh closure
    import jax

    jax.config.update("jax_compilation_cache_dir", "/tmp/jaxcache")
    jax.config.update("jax_persistent_cache_min_compile_time_secs", 0.0)
    jax.config.update("jax_persistent_cache_min_entry_size_bytes", 0)
except Exception:
    pass

BF = ml_dtypes.bfloat16

N_HEAD = 16
NOPE = 32
ROPE = 64
VDIM = 32
HD = NOPE + ROPE  # 96
WINDOW = 128
KEEP = 512
EPS = 1e-6
N_CORES = 4
HPC = 4  # heads per quad (the inner program unit)
QUADS = 2  # head-quads per core -> 8 heads per core
B, T, C = 2, 2048, 256
QT = 512  # q tile (free dim)
NJQ = T // QT  # 4 q tiles
NKB = T // 128  # 16 k blocks
SCALE = 1.0 / math.sqrt(HD)

# blob column offsets (all bf16, [128, NB]); per-quad weight sections are
# QW columns apart
XT = 0  # x^T, 2 cin chunks       [128, 2*T]
SEL = XT + 2 * T  # sel^T, 2 cin chunks     [128, 2*KEEP]
WSKN = SEL + 2 * KEEP  # Wsel_k nope, 2 chunks   [128, 2*128]
WSKR = WSKN + 256  # Wsel_k rope, 2 chunks   [128, 2*256]
WSV = WSKR + 512  # Wsel_v (gated), 2 chunks [128, 2*128]
WWKN = WSV + 256  # Wwin_k nope             [128, 2*128]
WWKR = WWKN + 256  # Wwin_k rope             [128, 2*256]
WWV = WWKR + 512  # Wwin_v (gated)          [128, 2*128]
QW = 2048  # per-quad stride of the WSKN..WWV block
WCQ = WSKN + QUADS * QW  # Wcq              [128, 2*96]
WCKV = WCQ + 192  # Wckv                    [128, 2*32]
WKR = WCKV + 64  # Wk_rope/N_HEAD          [128, 2*64]
NB = WKR + 128

# b96 columns ([96, N96]); per-quad stride QW96
WQN = 0  # Wq_nope (norm-folded)  [96, 128]
WQR = 128  # Wq_rope (norm-folded)  [96, 256]
QW96 = 384
N96 = QUADS * QW96

# b32 columns ([32, N32]); per-quad stride QW32 for the weight tail
C32 = 0  # cos^T [32, T]
S32 = T  # sin^T [32, T]
WKN = 2 * T  # Wk_nope (norm-folded) [32, 128]
WV = 2 * T + 128  # Wv (norm- and gate-folded) [32, 128]
QW32 = 256
N32 = 2 * T + QUADS * QW32

_CACHE = {}


# ---------------------------------------------------------------------------
# host-side helpers
# ---------------------------------------------------------------------------

def _freqs(t):
    f = 1.0 / 1e4 ** (np.arange(0, ROPE, 2, dtype=np.float32) / ROPE)
    ang = np.outer(np.arange(t, dtype=np.float32), f)
    return np.cos(ang).astype(np.float32), np.sin(ang).astype(np.float32)


# ---------------------------------------------------------------------------
# bass program (built once; identical for all 8 cores)
# ---------------------------------------------------------------------------

def _build_bass(legalize=True):
    import concourse.bass as bass
    import concourse.mybir as mybir
    import concourse.tile as tile

    f32 = mybir.dt.float32
    bf16 = mybir.dt.bfloat16
    EXP = mybir.ActivationFunctionType.Exp
    SQRT = mybir.ActivationFunctionType.Sqrt
    GE = mybir.AluOpType.is_ge

    nc = bass.Bass(target_bir_lowering=False, debug=False)

    d_blob = nc.declare_dram_parameter("blob", [128, NB], bf16, isOutput=False)
    d_b96 = nc.declare_dram_parameter("b96", [96, N96], bf16, isOutput=False)
    d_b32 = nc.declare_dram_parameter("b32", [32, N32], bf16, isOutput=False)
    d_out = nc.declare_dram_parameter("outT", [QUADS, 128, T], bf16,
                                      isOutput=True)

    def asl(base, s):  # absolute blob column slice
        return slice(base + s.start, base + s.stop)

    with tile.TileContext(nc) as tc:
        with (
            tc.tile_pool(name="const", bufs=1) as cpool,
            tc.tile_pool(name="big", bufs=1) as bpool,
            tc.tile_pool(name="pt", bufs=3) as ptpool,
            tc.tile_pool(name="sc", bufs=4) as scpool,
        ):
            _dma_engines = [nc.sync, nc.gpsimd, nc.scalar]
            _dma_rr = [0]

            def _dma(out, in_):
                eng = _dma_engines[_dma_rr[0] % len(_dma_engines)]
                _dma_rr[0] += 1
                eng.dma_start(out=out, in_=in_)

            s_blob = cpool.tile([128, NB], bf16, name="blob", tag="blob")
            s_b96 = cpool.tile([96, N96], bf16, name="b96", tag="b96")
            s_b32 = cpool.tile([32, N32], bf16, name="b32", tag="b32")

            # small/early-needed first; big x/sel tensors split for overlap
            _dma(s_b32[:, :], d_b32[:, :])
            _dma(s_b96[:, :], d_b96[:, :])
            _dma(s_blob[:, WSKN:NB], d_blob[:, WSKN:NB])
            _dma(s_blob[:, XT : XT + T], d_blob[:, XT : XT + T])
            _dma(s_blob[:, XT + T : XT + 2 * T], d_blob[:, XT + T : XT + 2 * T])
            _dma(s_blob[:, SEL : SEL + 2 * KEEP], d_blob[:, SEL : SEL + 2 * KEEP])

            def xT_ap(cc, ts_):
                return s_blob[:, asl(XT + cc * T, ts_)]

            # ---- rope tables [128, T] from compact [32, T] + sign folding ----
            c128 = cpool.tile([128, T], bf16, name="c128", tag="c128")
            s128 = cpool.tile([128, T], bf16, name="s128", tag="s128")
            for blk in range(4):
                rs = slice(32 * blk, 32 * blk + 32)
                nc.scalar.copy(c128[rs, :], s_b32[:, C32 : C32 + T])
                if blk % 2 == 0:
                    nc.vector.tensor_scalar_mul(
                        s128[rs, :], s_b32[:, S32 : S32 + T], -1.0)
                else:
                    nc.gpsimd.tensor_copy(s128[rs, :], s_b32[:, S32 : S32 + T])

            # ---- swapped rope weights built on device (column-half swap) ----
            wqrS = [cpool.tile([96, 256], bf16, name=f"wqrS{q}",
                               tag=f"wqrS{q}") for q in range(QUADS)]
            wskrS = [cpool.tile([128, 2, 256], bf16, name=f"wskrS{q}",
                                tag=f"wskrS{q}") for q in range(QUADS)]
            wwkrS = [cpool.tile([128, 2, 256], bf16, name=f"wwkrS{q}",
                                tag=f"wwkrS{q}") for q in range(QUADS)]
            wkrS = cpool.tile([128, 2, 64], bf16, tag="wkrS")
            _sw_rr = [0]

            def _sweng():
                eng = (nc.vector, nc.gpsimd, nc.scalar)[_sw_rr[0] % 3]
                _sw_rr[0] += 1
                return eng

            def swap_into(dst_ap, src_ap, nh):
                # both viewed [P, nh, 2, 32]; swap axis-2 halves
                dv = dst_ap.rearrange("p (h two c) -> p h two c", two=2, c=32)
                sv = src_ap.rearrange("p (h two c) -> p h two c", two=2, c=32)
                for half in range(2):
                    eng = _sweng()
                    (eng.tensor_copy if eng is not nc.scalar else eng.copy)(
                        dv[:, :, half, :], sv[:, :, 1 - half, :])

            for hq in range(QUADS):
                swap_into(wqrS[hq][:, :],
                          s_b96[:, hq * QW96 + WQR : hq * QW96 + WQR + 256], 4)
                for cc in range(2):
                    swap_into(
                        wskrS[hq][:, cc, :],
                        s_blob[:, hq * QW + WSKR + cc * 256 :
                               hq * QW + WSKR + cc * 256 + 256], 4)
                    swap_into(
                        wwkrS[hq][:, cc, :],
                        s_blob[:, hq * QW + WWKR + cc * 256 :
                               hq * QW + WWKR + cc * 256 + 256], 4)
            for cc in range(2):
                swap_into(wkrS[:, cc, :],
                          s_blob[:, WKR + cc * 64 : WKR + cc * 64 + 64], 1)

            # ---- constants for reductions/broadcasts ----
            ones96c = cpool.tile([96, 1], bf16, tag="o96c")
            ones32c = cpool.tile([32, 1], bf16, tag="o32c")
            ones96r = cpool.tile([1, 96], f32, tag="o96r")
            ones32r = cpool.tile([1, 32], f32, tag="o32r")
            epsc = cpool.tile([1, 1], f32, tag="epsc")
            nc.vector.memset(ones96c[:, :], 1.0)
            nc.vector.memset(ones32c[:, :], 1.0)
            nc.vector.memset(ones96r[:, :], 1.0)
            nc.vector.memset(ones32r[:, :], 1.0)
            nc.vector.memset(epsc[:, :], EPS)

            # ---- assembled per-head [96, h, T] q/k layouts ----
            cqT = bpool.tile([96, T], bf16)   # rms-normalized cq^T
            ckvT = bpool.tile([32, T], bf16)  # rms-normalized ckv^T
            q96 = bpool.tile([96, 4, T], bf16)
            k96 = bpool.tile([96, 4, T], bf16)     # branch 1 (kn | shared kr)
            ks96 = bpool.tile([96, 4, KEEP], bf16)  # branch 2
            kw96 = bpool.tile([96, 4, T], bf16)    # branch 3
            v1 = bpool.tile([128, NKB, 132], bf16)
            vs = bpool.tile([128, 4, 132], bf16)
            vw = bpool.tile([128, NKB, 132], bf16)
            acc = bpool.tile([128, T], f32)        # gated, normalized output
            outb = bpool.tile([128, T], bf16)

            with (
                tc.tile_pool(name="pp", bufs=2, space=bass.MemorySpace.PSUM) as pp,
                tc.tile_pool(name="sgp", bufs=2, space=bass.MemorySpace.PSUM) as sgp,
                tc.tile_pool(name="otp", bufs=2, space=bass.MemorySpace.PSUM) as otp,
            ):
                def rmsnorm_proj(dst, p, wcol, wwid, ones_col, ones_row, inv_n):
                    """dst[p, T] <- rms-normalized W^T x^T (tokens on free dim).
                    rms factor: ones-matmul column sum of squares -> Sqrt ACT
                    -> DVE reciprocal -> rank-1 f32 matmul broadcast."""
                    for t4 in range(NJQ):
                        ts_ = slice(t4 * QT, t4 * QT + QT)
                        ps = pp.tile([128, QT], f32, tag="p1",
                                     padded_shape=[128, QT])
                        for cc in range(2):
                            nc.tensor.matmul(
                                ps[0:p, :],
                                s_blob[:, wcol + cc * wwid : wcol + (cc + 1) * wwid],
                                xT_ap(cc, ts_), start=(cc == 0), stop=(cc == 1),
                            )
                        pre = scpool.tile([p, QT], bf16, tag="pre")
                        nc.scalar.copy(pre[:, :], ps[0:p, :])
                        sq = scpool.tile([p, QT], bf16, tag="sq")
                        nc.vector.tensor_mul(sq[:, :], pre[:, :], pre[:, :])
                        ps2 = pp.tile([1, QT], f32, tag="p1",
                                      padded_shape=[128, QT])
                        nc.tensor.matmul(ps2[:, :], ones_col[:, :], sq[:, :],
                                         start=True, stop=True)
                        srow = scpool.tile([1, QT], f32, tag="srow", bufs=2)
                        nc.scalar.activation(srow[:, :], ps2[:, :], SQRT,
                                             bias=epsc[:, :], scale=inv_n)
                        rin = scpool.tile([1, QT], f32, tag="rin", bufs=2)
                        nc.vector.reciprocal(rin[:, :], srow[:, :])
                        bc = pp.tile([128, QT], f32, tag="p1",
                                     padded_shape=[128, QT])
                        nc.tensor.matmul(bc[0:p, :], ones_row[:, 0:p],
                                         rin[:, :], start=True, stop=True)
                        nc.vector.tensor_mul(dst[:, ts_], pre[:, :], bc[0:p, :])

                def kr_build():
                    """k96[0:64, h, :] <- rope((x @ Wk_rope)/N_HEAD), all heads."""
                    for t4 in range(NJQ):
                        ts_ = slice(t4 * QT, t4 * QT + QT)
                        pr = pp.tile([128, QT], f32, tag="p1",
                                     padded_shape=[128, QT])
                        psw = pp.tile([128, QT], f32, tag="p1",
                                      padded_shape=[128, QT])
                        for cc in range(2):
                            nc.tensor.matmul(
                                pr[0:64, :],
                                s_blob[:, WKR + cc * 64 : WKR + cc * 64 + 64],
                                xT_ap(cc, ts_), start=(cc == 0), stop=(cc == 1),
                            )
                        for cc in range(2):
                            nc.tensor.matmul(
                                psw[0:64, :], wkrS[:, cc, :], xT_ap(cc, ts_),
                                start=(cc == 0), stop=(cc == 1),
                            )
                        t1 = scpool.tile([64, QT], bf16, tag="rt1")
                        t2 = scpool.tile([64, QT], bf16, tag="rt2")
                        nc.vector.tensor_mul(t1[:, :], pr[0:64, :],
                                             c128[0:64, ts_])
                        nc.vector.tensor_mul(t2[:, :], psw[0:64, :],
                                             s128[0:64, ts_])
                        nc.gpsimd.tensor_add(k96[0:64, 0, ts_], t1[:, :],
                                             t2[:, :])
                        for h in range(1, 4):
                            nc.scalar.copy(k96[0:64, h, ts_], k96[0:64, 0, ts_])

                def proj_nope(dest96, lhsW, rhs_of, tlen, nacc, eng=None):
                    """4-head nope projection, split per head into
                    dest96[64:96, h, ts]."""
                    step = min(tlen, QT)
                    for t4 in range(max(1, tlen // step)):
                        ts_ = slice(t4 * step, t4 * step + step)
                        ps = pp.tile([128, step], f32, tag="p1",
                                     padded_shape=[128, QT])
                        for cc in range(nacc):
                            nc.tensor.matmul(
                                ps[:], lhsW(cc), rhs_of(cc, ts_),
                                start=(cc == 0), stop=(cc == nacc - 1),
                            )
                        for h in range(4):
                            if eng == "act":
                                nc.scalar.copy(
                                    dest96[64:96, h, ts_],
                                    ps[32 * h : 32 * h + 32, :],
                                )
                            else:
                                nc.vector.tensor_copy(
                                    dest96[64:96, h, ts_],
                                    ps[32 * h : 32 * h + 32, :],
                                )

                def rope_proj(dest96, hpair, cos_sl, lhs_raw, lhs_sw,
                              rhs_list, tlen, ts_):
                    """Rope for one head-pair chunk; writes per-head rows
                    dest96[0:64, h, ts]."""
                    pr = pp.tile([128, tlen], f32, tag="p1",
                                 padded_shape=[128, QT])
                    psw = pp.tile([128, tlen], f32, tag="p1",
                                  padded_shape=[128, QT])
                    ncc = len(rhs_list)
                    for cc, rhs in enumerate(rhs_list):
                        nc.tensor.matmul(
                            pr[:], lhs_raw[cc], rhs,
                            start=(cc == 0), stop=(cc == ncc - 1),
                        )
                    for cc, rhs in enumerate(rhs_list):
                        nc.tensor.matmul(
                            psw[:], lhs_sw[cc], rhs,
                            start=(cc == 0), stop=(cc == ncc - 1),
                        )
                    t1 = scpool.tile([128, tlen], bf16, tag="rt1")
                    t2 = scpool.tile([128, tlen], bf16, tag="rt2")
                    nc.vector.tensor_mul(t1[:], pr[:], c128[:, cos_sl])
                    nc.vector.tensor_mul(t2[:], psw[:], s128[:, cos_sl])
                    for hi in range(2):
                        h = 2 * hpair + hi
                        hr = slice(64 * hi, 64 * hi + 64)
                        nc.gpsimd.tensor_add(
                            dest96[0:64, h, ts_], t1[hr, :], t2[hr, :]
                        )

                def rope_all(dest96, lhsW, lhsWS, rhs_of, tlen, nacc):
                    for j in range(2):
                        hs = slice(j * 128, j * 128 + 128)
                        step = min(tlen, QT)
                        for t4 in range(max(1, tlen // step)):
                            ts_ = slice(t4 * step, t4 * step + step)
                            rope_proj(
                                dest96, j, ts_,
                                [lhsW(cc, hs) for cc in range(nacc)],
                                [lhsWS(cc, hs) for cc in range(nacc)],
                                [rhs_of(cc, ts_) for cc in range(nacc)],
                                step, ts_,
                            )

                def v_tile(dest, nblk, lhs_fn, rhs_fn, nacc):
                    nc.vector.memset(dest[:, :, slice(32, 132, 33)], 1.0)
                    for tb in range(nblk):
                        ps = pp.tile([128, 128], f32, tag="p1",
                                     padded_shape=[128, QT])
                        for cc in range(nacc):
                            nc.tensor.matmul(
                                ps[:], lhs_fn(cc, tb), rhs_fn(cc),
                                start=(cc == 0), stop=(cc == nacc - 1),
                            )
                        nc.vector.tensor_copy(
                            dest[:, tb, :].rearrange(
                                "p (h c) -> p h c", h=4)[:, :, 0:32],
                            ps[:].rearrange("p (h c) -> p h c", h=4),
                        )

                def evict(jq, br, hp, ots):
                    """Normalize by the PSUM Z row and accumulate (gates are
                    folded into the V weights host-side)."""
                    oc = slice(jq * QT, jq * QT + QT)
                    for hi in range(2):
                        h = 2 * hp + hi
                        zi = scpool.tile([1, QT], f32, tag="zi", bufs=2)
                        nc.vector.reciprocal(zi[:, :], ots[hi][32:33, :])
                        zb = pp.tile([32, QT], f32, tag="p1",
                                     padded_shape=[128, QT])
                        nc.tensor.matmul(zb[:, :], ones32r[:, :], zi[:, :],
                                         start=True, stop=True)
                        # two PSUM srcs in one DVE op are illegal; stage zb
                        zbs = scpool.tile([32, QT], f32, tag="zbs", bufs=2)
                        nc.vector.tensor_copy(zbs[:, :], zb[:, :])
                        hr = slice(32 * h, 32 * h + 32)
                        ar = acc[hr, oc]
                        if br == 2:  # first writer of this acc region
                            nc.vector.tensor_mul(ar, ots[hi][0:32, :],
                                                 zbs[:, :])
                        else:
                            # tm band matches acc's partitions: walrus wants
                            # TT *inputs* on the same start partition
                            tm = scpool.tile([128, QT], f32, tag="tm", bufs=2)
                            nc.vector.tensor_mul(tm[hr, :], ots[hi][0:32, :],
                                                 zbs[:, :])
                            nc.gpsimd.tensor_add(ar, ar, tm[hr, :])

                def branch12(br, jqs):
                    kT = k96 if br == 1 else ks96
                    vt = v1 if br == 1 else vs
                    for jq in jqs:
                        nkb = 4 * (jq + 1) if br == 1 else 4
                        for hp in range(2):
                            ots = [
                                otp.tile([33, QT], f32, name=f"ot{i}",
                                         tag=f"ot{i}", bufs=1)
                                for i in range(2)
                            ]
                            for kb in range(nkb):
                                ksl = slice(kb * 128, kb * 128 + 128)
                                sg = sgp.tile([128, 1024], f32, tag="sg")
                                # on diagonal blocks only the causally-valid
                                # q columns [128i, QT) are ever consumed
                                off = (128 * (kb - 4 * jq)
                                       if br == 1 and kb >= 4 * jq else 0)
                                for hi in range(2):
                                    h = 2 * hp + hi
                                    nc.tensor.matmul(
                                        sg[:, hi * QT + off : hi * QT + QT],
                                        kT[:, h, ksl],
                                        q96[:, h,
                                            jq * QT + off : jq * QT + QT],
                                        start=True, stop=True,
                                    )
                                pt = ptpool.tile([128, 1024], bf16, tag="pt")
                                diag = br == 1 and kb >= 4 * jq
                                if diag:
                                    # exp only the causally-valid columns;
                                    # zero the rest, then mask the triangle
                                    i = kb - 4 * jq
                                    vq = slice(128 * i, QT)
                                    sgv = sg[:].rearrange(
                                        "p (h q) -> p h q", h=2)
                                    ptv = pt[:].rearrange(
                                        "p (h q) -> p h q", h=2)
                                    if i > 0:
                                        nc.gpsimd.memset(
                                            ptv[:, :, 0 : 128 * i], 0.0)
                                    nc.scalar.activation(
                                        ptv[:, :, vq], sgv[:, :, vq],
                                        EXP, scale=SCALE,
                                    )
                                    nc.gpsimd.affine_select(
                                        out=ptv[:, :, vq], in_=ptv[:, :, vq],
                                        compare_op=GE, fill=0.0,
                                        base=0,
                                        pattern=[[0, 2], [1, QT - 128 * i]],
                                        channel_multiplier=-1,
                                    )
                                else:
                                    nc.scalar.activation(
                                        pt[:], sg[:], EXP, scale=SCALE)
                                for hi in range(2):
                                    h = 2 * hp + hi
                                    nc.tensor.matmul(
                                        ots[hi][:],
                                        vt[:, kb, 33 * h : 33 * h + 33],
                                        pt[:, hi * QT : hi * QT + QT],
                                        start=(kb == 0), stop=(kb == nkb - 1),
                                    )
                            evict(jq, br, hp, ots)

                def branch3(jqs):
                    for jq in jqs:
                        for hp in range(2):
                            ots = [
                                otp.tile([33, QT], f32, name=f"ot{i}",
                                         tag=f"ot{i}", bufs=1)
                                for i in range(2)
                            ]
                            for qcp in range(2):  # pairs of 128-q chunks
                                sg = sgp.tile([128, 1024], f32, tag="sg")
                                for qcs in range(2):
                                    qb = 4 * jq + 2 * qcp + qcs
                                    qbs = slice(qb * 128, qb * 128 + 128)
                                    for hi in range(2):
                                        h = 2 * hp + hi
                                        for ki, kb in enumerate((qb - 1, qb)):
                                            col = slice(
                                                qcs * 512 + hi * 256 + ki * 128,
                                                qcs * 512 + hi * 256 + ki * 128
                                                + 128)
                                            if kb < 0:
                                                nc.vector.memset(
                                                    sg[:, col], 0.0)
                                                continue
                                            ksl = slice(kb * 128,
                                                        kb * 128 + 128)
                                            nc.tensor.matmul(
                                                sg[:, col], kw96[:, h, ksl],
                                                q96[:, h, qbs],
                                                start=True, stop=True,
                                            )
                                pt = ptpool.tile([128, 1024], bf16, tag="pt")
                                nc.scalar.activation(pt[:], sg[:], EXP,
                                                     scale=SCALE)
                                ptv = pt[:].rearrange("p (c q) -> p c q", c=8)
                                nc.gpsimd.affine_select(  # diag: col >= row
                                    out=ptv[:, slice(1, 8, 2)],
                                    in_=ptv[:, slice(1, 8, 2)],
                                    compare_op=GE, fill=0.0, base=0,
                                    pattern=[[0, 4], [1, 128]],
                                    channel_multiplier=-1,
                                )
                                nc.gpsimd.affine_select(  # prev: row > col
                                    out=ptv[:, slice(0, 8, 2)],
                                    in_=ptv[:, slice(0, 8, 2)],
                                    compare_op=GE, fill=0.0, base=-1,
                                    pattern=[[0, 4], [-1, 128]],
                                    channel_multiplier=1,
                                )
                                for qcs in range(2):
                                    qb = 4 * jq + 2 * qcp + qcs
                                    for hi in range(2):
                                        h = 2 * hp + hi
                                        for ki, kb in enumerate((qb - 1, qb)):
                                            if kb < 0:
                                                continue
                                            col = slice(
                                                qcs * 512 + hi * 256 + ki * 128,
                                                qcs * 512 + hi * 256 + ki * 128
                                                + 128)
                                            oc = slice(
                                                (2 * qcp + qcs) * 128,
                                                (2 * qcp + qcs) * 128 + 128)
                                            nc.tensor.matmul(
                                                ots[hi][:, oc],
                                                vw[:, kb,
                                                   33 * h : 33 * h + 33],
                                                pt[:, col],
                                                start=(kb == max(qb - 1, 0)),
                                                stop=(kb == qb),
                                            )
                            evict(jq, 3, hp, ots)

                # ---- emission: per-quad, q+b2 deps first (b2 is ACT-dense
                # and can start while the rest of the assembly runs on DVE).
                # Shared prep (cq/ckv/kr/tables) runs once; per-quad tiles
                # (q96, k96 nope rows, kw96, v*, acc) are rebuilt each quad.
                rmsnorm_proj(cqT, 96, WCQ, 96, ones96c, ones96r, 1.0 / 96)
                for hq in range(QUADS):
                    bo = hq * QW       # blob per-quad weight offset
                    b9 = hq * QW96     # b96 per-quad offset
                    b3 = hq * QW32     # b32 per-quad offset
                    proj_nope(q96,
                              lambda cc: s_b96[:, b9 + WQN : b9 + WQN + 128],
                              lambda cc, ts_: cqT[:, ts_], T, 1)
                    rope_all(q96,
                             lambda cc, hs: s_b96[:, asl(b9 + WQR, hs)],
                             lambda cc, hs: wqrS[hq][:, hs],
                             lambda cc, ts_: cqT[:, ts_], T, 1)
                    proj_nope(ks96,
                              lambda cc: s_blob[:, bo + WSKN + cc * 128 :
                                                bo + WSKN + cc * 128 + 128],
                              lambda cc, ts_: s_blob[:, asl(SEL + cc * KEEP, ts_)],
                              KEEP, 2)
                    rope_all(ks96,
                             lambda cc, hs: s_blob[:, asl(bo + WSKR + cc * 256, hs)],
                             lambda cc, hs: wskrS[hq][:, cc, hs],
                             lambda cc, ts_: s_blob[:, asl(SEL + cc * KEEP, ts_)],
                             KEEP, 2)
                    v_tile(
                        vs, 4,
                        lambda cc, tb: s_blob[:, SEL + cc * KEEP + tb * 128 :
                                              SEL + cc * KEEP + tb * 128 + 128],
                        lambda cc: s_blob[:, bo + WSV + cc * 128 :
                                          bo + WSV + cc * 128 + 128],
                        2,
                    )
                    branch12(2, range(NJQ))
                    if hq == 0:
                        # shared kv-path prep overlaps with branch 2
                        rmsnorm_proj(ckvT, 32, WCKV, 32, ones32c, ones32r,
                                     1.0 / 32)
                        kr_build()
                    proj_nope(k96, lambda cc: s_b32[:, b3 + WKN : b3 + WKN + 128],
                              lambda cc, ts_: ckvT[:, ts_], T, 1)
                    v_tile(
                        v1, NKB,
                        lambda cc, tb: ckvT[:, tb * 128 : tb * 128 + 128],
                        lambda cc: s_b32[:, b3 + WV : b3 + WV + 128], 1,
                    )
                    branch12(1, range(NJQ - 1))
                    proj_nope(kw96,
                              lambda cc: s_blob[:, bo + WWKN + cc * 128 :
                                                bo + WWKN + cc * 128 + 128],
                              lambda cc, ts_: xT_ap(cc, ts_), T, 2)
                    rope_all(kw96,
                             lambda cc, hs: s_blob[:, asl(bo + WWKR + cc * 256, hs)],
                             lambda cc, hs: wwkrS[hq][:, cc, hs],
                             lambda cc, ts_: xT_ap(cc, ts_), T, 2)
                    v_tile(
                        vw, NKB,
                        lambda cc, tb: xT_ap(cc, slice(tb * 128, tb * 128 + 128)),
                        lambda cc: s_blob[:, bo + WWV + cc * 128 :
                                          bo + WWV + cc * 128 + 128],
                        2,
                    )
                    branch3(range(NJQ))
                    branch12(1, [NJQ - 1])

                    # bf16 cast + output DMA for this quad (split by jq)
                    for jq in range(NJQ):
                        oc = slice(jq * QT, jq * QT + QT)
                        nc.scalar.copy(outb[:, oc], acc[:, oc])
                        _dma(d_out[hq, :, oc], outb[:, oc])

    if legalize:
        _legalize_pe_waits(nc, mybir)
    return nc


def _legalize_pe_waits(nc, mybir):
    """This walrus build encodes at most ONE sync-wait per compute
    instruction, but Tile emits up to 3. Split excess waits into standalone
    same-engine InstEventSemaphore waits placed immediately before the
    instruction (program point unchanged, so no deadlock risk)."""
    exempt = ("InstEventSemaphore", "InstNoOp",
              "InstUnconditionalBranch", "InstCall", "InstISA")
    for f in nc.m.functions:
        for bb in f.blocks:
            out = []
            changed = False
            for inst in bb.instructions:
                si = inst.sync_info
                tname = type(inst).__name__
                if si is not None and len(si.on_wait) > 1 and tname not in exempt:
                    for k, w in enumerate(si.on_wait[:-1]):
                        out.append(mybir.InstEventSemaphore(
                            name=f"{inst.name}-wsplit{k}",
                            engine=inst.engine,
                            ins=[], outs=[],
                            sync_info=mybir.SyncInfo(
                                on_wait=[w], on_update=[]),
                        ))
                    inst.sync_info = mybir.SyncInfo(
                        on_wait=[si.on_wait[-1]],
                        on_update=list(si.on_update),
                    )
                    changed = True
                out.append(inst)
            if changed:
                bb.instructions = out


def _get_nc():
    if "nc" not in _CACHE:
        nc = _build_bass()
        # bass2jax re-serializes the (frozen) BIR on every jit lowering;
        # memoize the bytes on this instance to keep per-call lower cheap
        try:
            raw = nc.to_json_bytes()
            nc.to_json_bytes = lambda: raw
        except Exception:
            pass
        _CACHE["nc"] = nc
    return _CACHE["nc"]


# ---------------------------------------------------------------------------
# host orchestration
# ---------------------------------------------------------------------------

def _prep_in_maps(inputs):
    x = np.asarray(inputs["x"], np.float32)
    cos, sin = _freqs(T)  # [T, 32] each
    c32 = np.ascontiguousarray(cos.T)  # [32, T]
    s32 = np.ascontiguousarray(sin.T)

    qnw = np.asarray(inputs["q_norm_w"], np.float32)
    kvw = np.asarray(inputs["kv_norm_w"], np.float32)
    Wqn_r = (np.asarray(inputs["Wq_nope"], np.float32)
             * qnw[:, None]).reshape(96, N_HEAD, NOPE)
    Wqr_r = (np.asarray(inputs["Wq_rope"], np.float32)
             * qnw[:, None]).reshape(96, N_HEAD, ROPE)
    Wkn_r = (np.asarray(inputs["Wk_nope"], np.float32)
             * kvw[:, None]).reshape(32, N_HEAD, NOPE)
    Wv_r = (np.asarray(inputs["Wv"], np.float32)
            * kvw[:, None]).reshape(32, N_HEAD, VDIM)
    Wsk_r = np.asarray(inputs["Wsel_k"], np.float32).reshape(C, N_HEAD, HD)
    Wsv_r = np.asarray(inputs["Wsel_v"], np.float32).reshape(C, N_HEAD, VDIM)
    Wwk_r = np.asarray(inputs["Wwin_k"], np.float32).reshape(C, N_HEAD, HD)
    Wwv_r = np.asarray(inputs["Wwin_v"], np.float32).reshape(C, N_HEAD, VDIM)
    Wkr = np.asarray(inputs["Wk_rope"], np.float32) / N_HEAD  # [C, 64]
    Wcq = np.asarray(inputs["Wcq"], np.float32)
    Wckv = np.asarray(inputs["Wckv"], np.float32)
    Wp = np.asarray(inputs["Wproj"], np.float32)

    per_batch = []
    for b in range(B):
        xb = x[b]
        glog = (xb @ np.asarray(inputs["Wgate"], np.float32)).mean(0)
        g = np.exp(glog - glog.max())
        g = (g / g.sum()).astype(np.float32)
        scores = (xb @ np.asarray(inputs["W_imp"], np.float32))[:, 0]
        idx = np.sort(np.argpartition(-scores, KEEP - 1)[:KEEP])
        per_batch.append((xb, xb[idx], g))

    in_maps = []
    for core in range(N_CORES):
        b, hg2 = divmod(core, N_CORES // B)
        xb, sel, g = per_batch[b]

        blob = np.zeros((128, NB), np.float32)

        def put(col, w):  # w [C, X] -> 2 cin chunks side by side
            Xw = w.shape[1]
            for cc in range(2):
                blob[:, col + cc * Xw : col + (cc + 1) * Xw] = \
                    w[cc * 128 : (cc + 1) * 128]

        put(XT, xb.T)
        put(SEL, sel.T)
        b96_parts, b32_parts = [], [c32, s32]
        for hq in range(QUADS):
            gq = hg2 * QUADS + hq  # global head-quad
            hsl = slice(gq * HPC, gq * HPC + HPC)
            bo = hq * QW
            put(bo + WSKN, Wsk_r[:, hsl, :NOPE].reshape(C, -1))
            put(bo + WSKR, Wsk_r[:, hsl, NOPE:].reshape(C, -1))
            put(bo + WSV, (Wsv_r[:, hsl] * g[1]).reshape(C, -1))
            put(bo + WWKN, Wwk_r[:, hsl, :NOPE].reshape(C, -1))
            put(bo + WWKR, Wwk_r[:, hsl, NOPE:].reshape(C, -1))
            put(bo + WWV, (Wwv_r[:, hsl] * g[2]).reshape(C, -1))
            b96_parts += [Wqn_r[:, hsl].reshape(96, -1),
                          Wqr_r[:, hsl].reshape(96, -1)]
            b32_parts += [Wkn_r[:, hsl].reshape(32, -1),
                          (Wv_r[:, hsl] * g[0]).reshape(32, -1)]
        put(WCQ, Wcq)
        put(WCKV, Wckv)
        put(WKR, Wkr)

        b96 = np.concatenate(b96_parts, 1)
        b32 = np.concatenate(b32_parts, 1)

        in_maps.append({
            "blob": blob.astype(BF),
            "b96": np.ascontiguousarray(b96).astype(BF),
            "b32": np.ascontiguousarray(b32).astype(BF),
        })
    return in_maps, Wp


def _run(inputs, trace=False):
    from concourse.bass_utils import run_bass_kernel_spmd

    nc = _get_nc()
    in_maps, Wp = _prep_in_maps(inputs)
    res = run_bass_kernel_spmd(nc, in_maps, list(range(N_CORES)), trace=trace)
    out = np.zeros((B, T, C), np.float32)
    for core in range(N_CORES):
        b, hg2 = divmod(core, N_CORES // B)
        accs = np.asarray(res.results[core]["outT"], np.float32)  # [2,128,T]
        for hq in range(QUADS):
            gq = hg2 * QUADS + hq
            out[b] += accs[hq].T @ Wp[gq * 128 : gq * 128 + 128]
    return out, res


def kernel(**inputs):
    out, _ = _run(inputs, trace=False)
    return out


# revision 26
# speedup vs baseline: 4.0094x; 1.1362x over previous
"""nn_Attn_9715216024104 — sparse attention (MLA + top-k select + sliding window).

Sharding: 8 cores = 2 batches x 4 head-groups (4 heads each). Each core runs
one Bass/Tile kernel computing its 4 heads' three attention branches
(S^T layout, exp softmax without max-subtraction — scores are <0.5 — with
ones-column-folded Z rows in the PV matmul).

This revision minimizes per-call host<->device traffic (the axon tunnel runs
at ~45 MB/s, so bytes dominate the wall clock):
- Inputs are packed into 3 DRAM params (blob[128,NB], b96, b32) instead of 20.
- cq/ckv RMS-norm and the shared roped kr are computed ON DEVICE from x
  (norm weights folded into downstream projections on the host; rms factor
  via ones-matmul column reduce + Sqrt activation + DVE reciprocal + rank-1
  f32 matmul partition-broadcast).
- Rope cos/sin tables ship compact as [32,T] and are replicated/sign-folded
  on device; "swapped" rope projection weights are built on device by
  column-half swaps instead of being shipped twice.
- The epilogue (divide by Z, gate, sum the 3 branches) runs ON DEVICE:
  branch gates are folded into the V projection weights on the host, evicts
  normalize by the PSUM Z row (DVE reciprocal + rank-1 broadcast) and
  accumulate into a f32 [128,T] tile; output is a single bf16 [128,T] per
  core. Host only applies Wproj per head-group and sums.
- kernel.py also enables JAX's persistent compilation cache so repeated
  run_bass_kernel_spmd calls skip the per-call XLA/NEFF rebuild.

Device layout notes:
- All matmul operands bf16 (rank-1 Z/rms broadcasts use f32); PSUM f32.
- Attention uses S^T tiles [k=128, q] so P^T feeds the PV matmul directly;
  V tiles carry a ones column so the PV matmul also produces Z rows.
- Rope is applied via duplicated "swapped" projection weights:
  rope(x) = x * cos + swap(x) * sgn*sin, with swap folded into a second
  matmul, so DVE only does 2 muls + 1 add.
- Causal / sliding-window masking is done on GPSIMD (affine_select zeroing
  of P^T after exp), keeping TensorE/ACT free of mask work.
"""

import math

import numpy as np
import ml_dtypes

try:  # persistent XLA compilation cache: the per-call jit is a fresh closure
    import jax

    jax.config.update("jax_compilation_cache_dir", "/tmp/jaxcache")
    jax.config.update("jax_persistent_cache_min_compile_time_secs", 0.0)
    jax.config.update("jax_persistent_cache_min_entry_size_bytes", 0)
except Exception:
    pass

BF = ml_dtypes.bfloat16

N_HEAD = 16
NOPE = 32
ROPE = 64
VDIM = 32
HD = NOPE + ROPE  # 96
WINDOW = 128
KEEP = 512
EPS = 1e-6
N_CORES = 2
HPC = 4  # heads per quad (the inner program unit)
QUADS = 4  # head-quads per core -> 16 heads per core
B, T, C = 2, 2048, 256
QT = 512  # q tile (free dim)
NJQ = T // QT  # 4 q tiles
NKB = T // 128  # 16 k blocks
SCALE = 1.0 / math.sqrt(HD)

# blob column offsets (all bf16, [128, NB]); per-quad weight sections are
# QW columns apart
XT = 0  # x^T, 2 cin chunks       [128, 2*T]
SEL = XT + 2 * T  # sel^T, 2 cin chunks     [128, 2*KEEP]
WSKN = SEL + 2 * KEEP  # Wsel_k nope, 2 chunks   [128, 2*128]
WSKR = WSKN + 256  # Wsel_k rope, 2 chunks   [128, 2*256]
WSV = WSKR + 512  # Wsel_v (gated), 2 chunks [128, 2*128]
WWKN = WSV + 256  # Wwin_k nope             [128, 2*128]
WWKR = WWKN + 256  # Wwin_k rope             [128, 2*256]
WWV = WWKR + 512  # Wwin_v (gated)          [128, 2*128]
QW = 2048  # per-quad stride of the WSKN..WWV block
WCQ = WSKN + QUADS * QW  # Wcq              [128, 2*96]
WCKV = WCQ + 192  # Wckv                    [128, 2*32]
WKR = WCKV + 64  # Wk_rope/N_HEAD          [128, 2*64]
NB = WKR + 128

# b96 columns ([96, N96]); per-quad stride QW96
WQN = 0  # Wq_nope (norm-folded)  [96, 128]
WQR = 128  # Wq_rope (norm-folded)  [96, 256]
QW96 = 384
N96 = QUADS * QW96

# b32 columns ([32, N32]); per-quad stride QW32 for the weight tail
C32 = 0  # cos^T [32, T]
S32 = T  # sin^T [32, T]
WKN = 2 * T  # Wk_nope (norm-folded) [32, 128]
WV = 2 * T + 128  # Wv (norm- and gate-folded) [32, 128]
QW32 = 256
N32 = 2 * T + QUADS * QW32

_CACHE = {}


# ---------------------------------------------------------------------------
# host-side helpers
# ---------------------------------------------------------------------------

def _freqs(t):
    f = 1.0 / 1e4 ** (np.arange(0, ROPE, 2, dtype=np.float32) / ROPE)
    ang = np.outer(np.arange(t, dtype=np.float32), f)
    return np.cos(ang).astype(np.float32), np.sin(ang).astype(np.float32)


# ---------------------------------------------------------------------------
# bass program (built once; identical for all 8 cores)
# ---------------------------------------------------------------------------

def _build_bass(legalize=True):
    import concourse.bass as bass
    import concourse.mybir as mybir
    import concourse.tile as tile

    f32 = mybir.dt.float32
    bf16 = mybir.dt.bfloat16
    EXP = mybir.ActivationFunctionType.Exp
    SQRT = mybir.ActivationFunctionType.Sqrt
    GE = mybir.AluOpType.is_ge

    nc = bass.Bass(target_bir_lowering=False, debug=False)

    d_blob = nc.declare_dram_parameter("blob", [128, NB], bf16, isOutput=False)
    d_b96 = nc.declare_dram_parameter("b96", [96, N96], bf16, isOutput=False)
    d_b32 = nc.declare_dram_parameter("b32", [32, N32], bf16, isOutput=False)
    d_out = nc.declare_dram_parameter("outT", [QUADS, 128, T], bf16,
                                      isOutput=True)

    def asl(base, s):  # absolute blob column slice
        return slice(base + s.start, base + s.stop)

    with tile.TileContext(nc) as tc:
        with (
            tc.tile_pool(name="const", bufs=1) as cpool,
            tc.tile_pool(name="big", bufs=1) as bpool,
            tc.tile_pool(name="pt", bufs=3) as ptpool,
            tc.tile_pool(name="sc", bufs=4) as scpool,
        ):
            _dma_engines = [nc.sync, nc.gpsimd, nc.scalar]
            _dma_rr = [0]

            def _dma(out, in_):
                eng = _dma_engines[_dma_rr[0] % len(_dma_engines)]
                _dma_rr[0] += 1
                eng.dma_start(out=out, in_=in_)

            s_blob = cpool.tile([128, NB], bf16, name="blob", tag="blob")
            s_b96 = cpool.tile([96, N96], bf16, name="b96", tag="b96")
            s_b32 = cpool.tile([32, N32], bf16, name="b32", tag="b32")

            # small/early-needed first; big x/sel tensors split for overlap
            _dma(s_b32[:, :], d_b32[:, :])
            _dma(s_b96[:, :], d_b96[:, :])
            _dma(s_blob[:, WSKN:NB], d_blob[:, WSKN:NB])
            _dma(s_blob[:, XT : XT + T], d_blob[:, XT : XT + T])
            _dma(s_blob[:, XT + T : XT + 2 * T], d_blob[:, XT + T : XT + 2 * T])
            _dma(s_blob[:, SEL : SEL + 2 * KEEP], d_blob[:, SEL : SEL + 2 * KEEP])

            def xT_ap(cc, ts_):
                return s_blob[:, asl(XT + cc * T, ts_)]

            # ---- rope tables [128, T] from compact [32, T] + sign folding ----
            c128 = cpool.tile([128, T], bf16, name="c128", tag="c128")
            s128 = cpool.tile([128, T], bf16, name="s128", tag="s128")
            for blk in range(4):
                rs = slice(32 * blk, 32 * blk + 32)
                nc.scalar.copy(c128[rs, :], s_b32[:, C32 : C32 + T])
                if blk % 2 == 0:
                    nc.vector.tensor_scalar_mul(
                        s128[rs, :], s_b32[:, S32 : S32 + T], -1.0)
                else:
                    nc.gpsimd.tensor_copy(s128[rs, :], s_b32[:, S32 : S32 + T])

            # ---- swapped rope weights built on device (column-half swap) ----
            wqrS = [cpool.tile([96, 256], bf16, name=f"wqrS{q}",
                               tag=f"wqrS{q}") for q in range(QUADS)]
            wskrS = [cpool.tile([128, 2, 256], bf16, name=f"wskrS{q}",
                                tag=f"wskrS{q}") for q in range(QUADS)]
            wwkrS = [cpool.tile([128, 2, 256], bf16, name=f"wwkrS{q}",
                                tag=f"wwkrS{q}") for q in range(QUADS)]
            wkrS = cpool.tile([128, 2, 64], bf16, tag="wkrS")
            _sw_rr = [0]

            def _sweng():
                eng = (nc.vector, nc.gpsimd, nc.scalar)[_sw_rr[0] % 3]
                _sw_rr[0] += 1
                return eng

            def swap_into(dst_ap, src_ap, nh):
                # both viewed [P, nh, 2, 32]; swap axis-2 halves
                dv = dst_ap.rearrange("p (h two c) -> p h two c", two=2, c=32)
                sv = src_ap.rearrange("p (h two c) -> p h two c", two=2, c=32)
                for half in range(2):
                    eng = _sweng()
                    (eng.tensor_copy if eng is not nc.scalar else eng.copy)(
                        dv[:, :, half, :], sv[:, :, 1 - half, :])

            for hq in range(QUADS):
                swap_into(wqrS[hq][:, :],
                          s_b96[:, hq * QW96 + WQR : hq * QW96 + WQR + 256], 4)
                for cc in range(2):
                    swap_into(
                        wskrS[hq][:, cc, :],
                        s_blob[:, hq * QW + WSKR + cc * 256 :
                               hq * QW + WSKR + cc * 256 + 256], 4)
                    swap_into(
                        wwkrS[hq][:, cc, :],
                        s_blob[:, hq * QW + WWKR + cc * 256 :
                               hq * QW + WWKR + cc * 256 + 256], 4)
            for cc in range(2):
                swap_into(wkrS[:, cc, :],
                          s_blob[:, WKR + cc * 64 : WKR + cc * 64 + 64], 1)

            # ---- constants for reductions/broadcasts ----
            ones96c = cpool.tile([96, 1], bf16, tag="o96c")
            ones32c = cpool.tile([32, 1], bf16, tag="o32c")
            ones96r = cpool.tile([1, 96], f32, tag="o96r")
            ones32r = cpool.tile([1, 32], f32, tag="o32r")
            epsc = cpool.tile([1, 1], f32, tag="epsc")
            nc.vector.memset(ones96c[:, :], 1.0)
            nc.vector.memset(ones32c[:, :], 1.0)
            nc.vector.memset(ones96r[:, :], 1.0)
            nc.vector.memset(ones32r[:, :], 1.0)
            nc.vector.memset(epsc[:, :], EPS)

            # ---- assembled per-head [96, h, T] q/k layouts ----
            cqT = bpool.tile([96, T], bf16)   # rms-normalized cq^T
            ckvT = bpool.tile([32, T], bf16)  # rms-normalized ckv^T
            q96 = bpool.tile([96, 4, T], bf16)
            k96 = bpool.tile([96, 4, T], bf16)     # branch 1 (kn | shared kr)
            ks96 = bpool.tile([96, 4, KEEP], bf16)  # branch 2
            kw96 = bpool.tile([96, 4, T], bf16)    # branch 3
            v1 = bpool.tile([128, NKB, 132], bf16)
            vs = bpool.tile([128, 4, 132], bf16)
            vw = bpool.tile([128, NKB, 132], bf16)
            acc = bpool.tile([128, T], f32)        # gated, normalized output
            outb = bpool.tile([128, T], bf16)

            with (
                tc.tile_pool(name="pp", bufs=2, space=bass.MemorySpace.PSUM) as pp,
                tc.tile_pool(name="sgp", bufs=2, space=bass.MemorySpace.PSUM) as sgp,
                tc.tile_pool(name="otp", bufs=2, space=bass.MemorySpace.PSUM) as otp,
            ):
                def rmsnorm_proj(dst, p, wcol, wwid, ones_col, ones_row, inv_n):
                    """dst[p, T] <- rms-normalized W^T x^T (tokens on free dim).
                    rms factor: ones-matmul column sum of squares -> Sqrt ACT
                    -> DVE reciprocal -> rank-1 f32 matmul broadcast."""
                    for t4 in range(NJQ):
                        ts_ = slice(t4 * QT, t4 * QT + QT)
                        ps = pp.tile([128, QT], f32, tag="p1",
                                     padded_shape=[128, QT])
                        for cc in range(2):
                            nc.tensor.matmul(
                                ps[0:p, :],
                                s_blob[:, wcol + cc * wwid : wcol + (cc + 1) * wwid],
                                xT_ap(cc, ts_), start=(cc == 0), stop=(cc == 1),
                            )
                        pre = scpool.tile([p, QT], bf16, tag="pre")
                        nc.scalar.copy(pre[:, :], ps[0:p, :])
                        sq = scpool.tile([p, QT], bf16, tag="sq")
                        nc.vector.tensor_mul(sq[:, :], pre[:, :], pre[:, :])
                        ps2 = pp.tile([1, QT], f32, tag="p1",
                                      padded_shape=[128, QT])
                        nc.tensor.matmul(ps2[:, :], ones_col[:, :], sq[:, :],
                                         start=True, stop=True)
                        srow = scpool.tile([1, QT], f32, tag="srow", bufs=2)
                        nc.scalar.activation(srow[:, :], ps2[:, :], SQRT,
                                             bias=epsc[:, :], scale=inv_n)
                        rin = scpool.tile([1, QT], f32, tag="rin", bufs=2)
                        nc.vector.reciprocal(rin[:, :], srow[:, :])
                        bc = pp.tile([128, QT], f32, tag="p1",
                                     padded_shape=[128, QT])
                        nc.tensor.matmul(bc[0:p, :], ones_row[:, 0:p],
                                         rin[:, :], start=True, stop=True)
                        nc.vector.tensor_mul(dst[:, ts_], pre[:, :], bc[0:p, :])

                def kr_build():
                    """k96[0:64, h, :] <- rope((x @ Wk_rope)/N_HEAD), all heads."""
                    for t4 in range(NJQ):
                        ts_ = slice(t4 * QT, t4 * QT + QT)
                        pr = pp.tile([128, QT], f32, tag="p1",
                                     padded_shape=[128, QT])
                        psw = pp.tile([128, QT], f32, tag="p1",
                                      padded_shape=[128, QT])
                        for cc in range(2):
                            nc.tensor.matmul(
                                pr[0:64, :],
                                s_blob[:, WKR + cc * 64 : WKR + cc * 64 + 64],
                                xT_ap(cc, ts_), start=(cc == 0), stop=(cc == 1),
                            )
                        for cc in range(2):
                            nc.tensor.matmul(
                                psw[0:64, :], wkrS[:, cc, :], xT_ap(cc, ts_),
                                start=(cc == 0), stop=(cc == 1),
                            )
                        t1 = scpool.tile([64, QT], bf16, tag="rt1")
                        t2 = scpool.tile([64, QT], bf16, tag="rt2")
                        nc.vector.tensor_mul(t1[:, :], pr[0:64, :],
                                             c128[0:64, ts_])
                        nc.vector.tensor_mul(t2[:, :], psw[0:64, :],
                                             s128[0:64, ts_])
                        nc.gpsimd.tensor_add(k96[0:64, 0, ts_], t1[:, :],
                                             t2[:, :])
                        for h in range(1, 4):
                            nc.scalar.copy(k96[0:64, h, ts_], k96[0:64, 0, ts_])

                def proj_nope(dest96, lhsW, rhs_of, tlen, nacc, eng=None):
                    """4-head nope projection, split per head into
                    dest96[64:96, h, ts]."""
                    step = min(tlen, QT)
                    for t4 in range(max(1, tlen // step)):
                        ts_ = slice(t4 * step, t4 * step + step)
                        ps = pp.tile([128, step], f32, tag="p1",
                                     padded_shape=[128, QT])
                        for cc in range(nacc):
                            nc.tensor.matmul(
                                ps[:], lhsW(cc), rhs_of(cc, ts_),
                                start=(cc == 0), stop=(cc == nacc - 1),
                            )
                        for h in range(4):
                            if eng == "act":
                                nc.scalar.copy(
                                    dest96[64:96, h, ts_],
                                    ps[32 * h : 32 * h + 32, :],
                                )
                            else:
                                nc.vector.tensor_copy(
                                    dest96[64:96, h, ts_],
                                    ps[32 * h : 32 * h + 32, :],
                                )

                def rope_proj(dest96, hpair, cos_sl, lhs_raw, lhs_sw,
                              rhs_list, tlen, ts_):
                    """Rope for one head-pair chunk; writes per-head rows
                    dest96[0:64, h, ts]."""
                    pr = pp.tile([128, tlen], f32, tag="p1",
                                 padded_shape=[128, QT])
                    psw = pp.tile([128, tlen], f32, tag="p1",
                                  padded_shape=[128, QT])
                    ncc = len(rhs_list)
                    for cc, rhs in enumerate(rhs_list):
                        nc.tensor.matmul(
                            pr[:], lhs_raw[cc], rhs,
                            start=(cc == 0), stop=(cc == ncc - 1),
                        )
                    for cc, rhs in enumerate(rhs_list):
                        nc.tensor.matmul(
                            psw[:], lhs_sw[cc], rhs,
                            start=(cc == 0), stop=(cc == ncc - 1),
                        )
                    t1 = scpool.tile([128, tlen], bf16, tag="rt1")
                    t2 = scpool.tile([128, tlen], bf16, tag="rt2")
                    nc.vector.tensor_mul(t1[:], pr[:], c128[:, cos_sl])
                    nc.vector.tensor_mul(t2[:], psw[:], s128[:, cos_sl])
                    for hi in range(2):
                        h = 2 * hpair + hi
                        hr = slice(64 * hi, 64 * hi + 64)
                        nc.gpsimd.tensor_add(
                            dest96[0:64, h, ts_], t1[hr, :], t2[hr, :]
                        )

                def rope_all(dest96, lhsW, lhsWS, rhs_of, tlen, nacc):
                    for j in range(2):
                        hs = slice(j * 128, j * 128 + 128)
                        step = min(tlen, QT)
                        for t4 in range(max(1, tlen // step)):
                            ts_ = slice(t4 * step, t4 * step + step)
                            rope_proj(
                                dest96, j, ts_,
                                [lhsW(cc, hs) for cc in range(nacc)],
                                [lhsWS(cc, hs) for cc in range(nacc)],
                                [rhs_of(cc, ts_) for cc in range(nacc)],
                                step, ts_,
                            )

                def v_tile(dest, nblk, lhs_fn, rhs_fn, nacc):
                    nc.vector.memset(dest[:, :, slice(32, 132, 33)], 1.0)
                    for tb in range(nblk):
                        ps = pp.tile([128, 128], f32, tag="p1",
                                     padded_shape=[128, QT])
                        for cc in range(nacc):
                            nc.tensor.matmul(
                                ps[:], lhs_fn(cc, tb), rhs_fn(cc),
                                start=(cc == 0), stop=(cc == nacc - 1),
                            )
                        nc.vector.tensor_copy(
                            dest[:, tb, :].rearrange(
                                "p (h c) -> p h c", h=4)[:, :, 0:32],
                            ps[:].rearrange("p (h c) -> p h c", h=4),
                        )

                def evict(jq, br, hp, ots):
                    """Normalize by the PSUM Z row and accumulate (gates are
                    folded into the V weights host-side)."""
                    oc = slice(jq * QT, jq * QT + QT)
                    for hi in range(2):
                        h = 2 * hp + hi
                        zi = scpool.tile([1, QT], f32, tag="zi", bufs=2)
                        nc.vector.reciprocal(zi[:, :], ots[hi][32:33, :])
                        zb = pp.tile([32, QT], f32, tag="p1",
                                     padded_shape=[128, QT])
                        nc.tensor.matmul(zb[:, :], ones32r[:, :], zi[:, :],
                                         start=True, stop=True)
                        # two PSUM srcs in one DVE op are illegal; stage zb
                        zbs = scpool.tile([32, QT], f32, tag="zbs", bufs=2)
                        nc.vector.tensor_copy(zbs[:, :], zb[:, :])
                        hr = slice(32 * h, 32 * h + 32)
                        ar = acc[hr, oc]
                        if br == 2:  # first writer of this acc region
                            nc.vector.tensor_mul(ar, ots[hi][0:32, :],
                                                 zbs[:, :])
                        else:
                            # tm band matches acc's partitions: walrus wants
                            # TT *inputs* on the same start partition
                            tm = scpool.tile([128, QT], f32, tag="tm", bufs=2)
                            nc.vector.tensor_mul(tm[hr, :], ots[hi][0:32, :],
                                                 zbs[:, :])
                            nc.gpsimd.tensor_add(ar, ar, tm[hr, :])

                def branch12(br, jqs):
                    kT = k96 if br == 1 else ks96
                    vt = v1 if br == 1 else vs
                    for jq in jqs:
                        nkb = 4 * (jq + 1) if br == 1 else 4
                        for hp in range(2):
                            ots = [
                                otp.tile([33, QT], f32, name=f"ot{i}",
                                         tag=f"ot{i}", bufs=1)
                                for i in range(2)
                            ]
                            for kb in range(nkb):
                                ksl = slice(kb * 128, kb * 128 + 128)
                                sg = sgp.tile([128, 1024], f32, tag="sg")
                                # on diagonal blocks only the causally-valid
                                # q columns [128i, QT) are ever consumed
                                off = (128 * (kb - 4 * jq)
                                       if br == 1 and kb >= 4 * jq else 0)
                                for hi in range(2):
                                    h = 2 * hp + hi
                                    nc.tensor.matmul(
                                        sg[:, hi * QT + off : hi * QT + QT],
                                        kT[:, h, ksl],
                                        q96[:, h,
                                            jq * QT + off : jq * QT + QT],
                                        start=True, stop=True,
                                    )
                                pt = ptpool.tile([128, 1024], bf16, tag="pt")
                                diag = br == 1 and kb >= 4 * jq
                                if diag:
                                    # exp only the causally-valid columns;
                                    # zero the rest, then mask the triangle
                                    i = kb - 4 * jq
                                    vq = slice(128 * i, QT)
                                    sgv = sg[:].rearrange(
                                        "p (h q) -> p h q", h=2)
                                    ptv = pt[:].rearrange(
                                        "p (h q) -> p h q", h=2)
                                    if i > 0:
                                        nc.gpsimd.memset(
                                            ptv[:, :, 0 : 128 * i], 0.0)
                                    nc.scalar.activation(
                                        ptv[:, :, vq], sgv[:, :, vq],
                                        EXP, scale=SCALE,
                                    )
                                    nc.gpsimd.affine_select(
                                        out=ptv[:, :, vq], in_=ptv[:, :, vq],
                                        compare_op=GE, fill=0.0,
                                        base=0,
                                        pattern=[[0, 2], [1, QT - 128 * i]],
                                        channel_multiplier=-1,
                                    )
                                else:
                                    nc.scalar.activation(
                                        pt[:], sg[:], EXP, scale=SCALE)
                                for hi in range(2):
                                    h = 2 * hp + hi
                                    nc.tensor.matmul(
                                        ots[hi][:],
                                        vt[:, kb, 33 * h : 33 * h + 33],
                                        pt[:, hi * QT : hi * QT + QT],
                                        start=(kb == 0), stop=(kb == nkb - 1),
                                    )
                            evict(jq, br, hp, ots)

                def branch3(jqs):
                    for jq in jqs:
                        for hp in range(2):
                            ots = [
                                otp.tile([33, QT], f32, name=f"ot{i}",
                                         tag=f"ot{i}", bufs=1)
                                for i in range(2)
                            ]
                            for qcp in range(2):  # pairs of 128-q chunks
                                sg = sgp.tile([128, 1024], f32, tag="sg")
                                for qcs in range(2):
                                    qb = 4 * jq + 2 * qcp + qcs
                                    qbs = slice(qb * 128, qb * 128 + 128)
                                    for hi in range(2):
                                        h = 2 * hp + hi
                                        for ki, kb in enumerate((qb - 1, qb)):
                                            col = slice(
                                                qcs * 512 + hi * 256 + ki * 128,
                                                qcs * 512 + hi * 256 + ki * 128
                                                + 128)
                                            if kb < 0:
                                                nc.vector.memset(
                                                    sg[:, col], 0.0)
                                                continue
                                            ksl = slice(kb * 128,
                                                        kb * 128 + 128)
                                            nc.tensor.matmul(
                                                sg[:, col], kw96[:, h, ksl],
                                                q96[:, h, qbs],
                                                start=True, stop=True,
                                            )
                                pt = ptpool.tile([128, 1024], bf16, tag="pt")
                                nc.scalar.activation(pt[:], sg[:], EXP,
                                                     scale=SCALE)
                                ptv = pt[:].rearrange("p (c q) -> p c q", c=8)
                                nc.gpsimd.affine_select(  # diag: col >= row
                                    out=ptv[:, slice(1, 8, 2)],
                                    in_=ptv[:, slice(1, 8, 2)],
                                    compare_op=GE, fill=0.0, base=0,
                                    pattern=[[0, 4], [1, 128]],
                                    channel_multiplier=-1,
                                )
                                nc.gpsimd.affine_select(  # prev: row > col
                                    out=ptv[:, slice(0, 8, 2)],
                                    in_=ptv[:, slice(0, 8, 2)],
                                    compare_op=GE, fill=0.0, base=-1,
                                    pattern=[[0, 4], [-1, 128]],
                                    channel_multiplier=1,
                                )
                                for qcs in range(2):
                                    qb = 4 * jq + 2 * qcp + qcs
                                    for hi in range(2):
                                        h = 2 * hp + hi
                                        for ki, kb in enumerate((qb - 1, qb)):
                                            if kb < 0:
                                                continue
                                            col = slice(
                                                qcs * 512 + hi * 256 + ki * 128,
                                                qcs * 512 + hi * 256 + ki * 128
                                                + 128)
                                            oc = slice(
                                                (2 * qcp + qcs) * 128,
                                                (2 * qcp + qcs) * 128 + 128)
                                            nc.tensor.matmul(
                                                ots[hi][:, oc],
                                                vw[:, kb,
                                                   33 * h : 33 * h + 33],
                                                pt[:, col],
                                                start=(kb == max(qb - 1, 0)),
                                                stop=(kb == qb),
                                            )
                            evict(jq, 3, hp, ots)

                # ---- emission: per-quad, q+b2 deps first (b2 is ACT-dense
                # and can start while the rest of the assembly runs on DVE).
                # Shared prep (cq/ckv/kr/tables) runs once; per-quad tiles
                # (q96, k96 nope rows, kw96, v*, acc) are rebuilt each quad.
                rmsnorm_proj(cqT, 96, WCQ, 96, ones96c, ones96r, 1.0 / 96)
                for hq in range(QUADS):
                    bo = hq * QW       # blob per-quad weight offset
                    b9 = hq * QW96     # b96 per-quad offset
                    b3 = hq * QW32     # b32 per-quad offset
                    proj_nope(q96,
                              lambda cc: s_b96[:, b9 + WQN : b9 + WQN + 128],
                              lambda cc, ts_: cqT[:, ts_], T, 1)
                    rope_all(q96,
                             lambda cc, hs: s_b96[:, asl(b9 + WQR, hs)],
                             lambda cc, hs: wqrS[hq][:, hs],
                             lambda cc, ts_: cqT[:, ts_], T, 1)
                    proj_nope(ks96,
                              lambda cc: s_blob[:, bo + WSKN + cc * 128 :
                                                bo + WSKN + cc * 128 + 128],
                              lambda cc, ts_: s_blob[:, asl(SEL + cc * KEEP, ts_)],
                              KEEP, 2)
                    rope_all(ks96,
                             lambda cc, hs: s_blob[:, asl(bo + WSKR + cc * 256, hs)],
                             lambda cc, hs: wskrS[hq][:, cc, hs],
                             lambda cc, ts_: s_blob[:, asl(SEL + cc * KEEP, ts_)],
                             KEEP, 2)
                    v_tile(
                        vs, 4,
                        lambda cc, tb: s_blob[:, SEL + cc * KEEP + tb * 128 :
                                              SEL + cc * KEEP + tb * 128 + 128],
                        lambda cc: s_blob[:, bo + WSV + cc * 128 :
                                          bo + WSV + cc * 128 + 128],
                        2,
                    )
                    branch12(2, range(NJQ))
                    if hq == 0:
                        # shared kv-path prep overlaps with branch 2
                        rmsnorm_proj(ckvT, 32, WCKV, 32, ones32c, ones32r,
                                     1.0 / 32)
                        kr_build()
                    proj_nope(k96, lambda cc: s_b32[:, b3 + WKN : b3 + WKN + 128],
                              lambda cc, ts_: ckvT[:, ts_], T, 1)
                    v_tile(
                        v1, NKB,
                        lambda cc, tb: ckvT[:, tb * 128 : tb * 128 + 128],
                        lambda cc: s_b32[:, b3 + WV : b3 + WV + 128], 1,
                    )
                    branch12(1, range(NJQ - 1))
                    proj_nope(kw96,
                              lambda cc: s_blob[:, bo + WWKN + cc * 128 :
                                                bo + WWKN + cc * 128 + 128],
                              lambda cc, ts_: xT_ap(cc, ts_), T, 2)
                    rope_all(kw96,
                             lambda cc, hs: s_blob[:, asl(bo + WWKR + cc * 256, hs)],
                             lambda cc, hs: wwkrS[hq][:, cc, hs],
                             lambda cc, ts_: xT_ap(cc, ts_), T, 2)
                    v_tile(
                        vw, NKB,
                        lambda cc, tb: xT_ap(cc, slice(tb * 128, tb * 128 + 128)),
                        lambda cc: s_blob[:, bo + WWV + cc * 128 :
                                          bo + WWV + cc * 128 + 128],
                        2,
                    )
                    branch3(range(NJQ))
                    branch12(1, [NJQ - 1])

                    # bf16 cast + output DMA for this quad (split by jq)
                    for jq in range(NJQ):
                        oc = slice(jq * QT, jq * QT + QT)
                        nc.scalar.copy(outb[:, oc], acc[:, oc])
                        _dma(d_out[hq, :, oc], outb[:, oc])

    if legalize:
        _legalize_pe_waits(nc, mybir)
    return nc


def _legalize_pe_waits(nc, mybir):
    """This walrus build encodes at most ONE sync-wait per compute
    instruction, but Tile emits up to 3. Split excess waits into standalone
    same-engine InstEventSemaphore waits placed immediately before the
    instruction (program point unchanged, so no deadlock risk)."""
    exempt = ("InstEventSemaphore", "InstNoOp",
              "InstUnconditionalBranch", "InstCall", "InstISA")
    for f in nc.m.functions:
        for bb in f.blocks:
            out = []
            changed = False
            for inst in bb.instructions:
                si = inst.sync_info
                tname = type(inst).__name__
                if si is not None and len(si.on_wait) > 1 and tname not in exempt:
                    for k, w in enumerate(si.on_wait[:-1]):
                        out.append(mybir.InstEventSemaphore(
                            name=f"{inst.name}-wsplit{k}",
                            engine=inst.engine,
                            ins=[], outs=[],
                            sync_info=mybir.SyncInfo(
                                on_wait=[w], on_update=[]),
                        ))
                    inst.sync_info = mybir.SyncInfo(
                        on_wait=[si.on_wait[-1]],
                        on_update=list(si.on_update),
                    )
                    changed = True
                out.append(inst)
            if changed:
                bb.instructions = out


def _get_nc():
    if "nc" not in _CACHE:
        nc = _build_bass()
        # bass2jax re-serializes the (frozen) BIR on every jit lowering;
        # memoize the bytes on this instance to keep per-call lower cheap
        try:
            raw = nc.to_json_bytes()
            nc.to_json_bytes = lambda: raw
        except Exception:
            pass
        _CACHE["nc"] = nc
    return _CACHE["nc"]


# ---------------------------------------------------------------------------
# host orchestration
# ---------------------------------------------------------------------------

def _prep_in_maps(inputs):
    x = np.asarray(inputs["x"], np.float32)
    cos, sin = _freqs(T)  # [T, 32] each
    c32 = np.ascontiguousarray(cos.T)  # [32, T]
    s32 = np.ascontiguousarray(sin.T)

    qnw = np.asarray(inputs["q_norm_w"], np.float32)
    kvw = np.asarray(inputs["kv_norm_w"], np.float32)
    Wqn_r = (np.asarray(inputs["Wq_nope"], np.float32)
             * qnw[:, None]).reshape(96, N_HEAD, NOPE)
    Wqr_r = (np.asarray(inputs["Wq_rope"], np.float32)
             * qnw[:, None]).reshape(96, N_HEAD, ROPE)
    Wkn_r = (np.asarray(inputs["Wk_nope"], np.float32)
             * kvw[:, None]).reshape(32, N_HEAD, NOPE)
    Wv_r = (np.asarray(inputs["Wv"], np.float32)
            * kvw[:, None]).reshape(32, N_HEAD, VDIM)
    Wsk_r = np.asarray(inputs["Wsel_k"], np.float32).reshape(C, N_HEAD, HD)
    Wsv_r = np.asarray(inputs["Wsel_v"], np.float32).reshape(C, N_HEAD, VDIM)
    Wwk_r = np.asarray(inputs["Wwin_k"], np.float32).reshape(C, N_HEAD, HD)
    Wwv_r = np.asarray(inputs["Wwin_v"], np.float32).reshape(C, N_HEAD, VDIM)
    Wkr = np.asarray(inputs["Wk_rope"], np.float32) / N_HEAD  # [C, 64]
    Wcq = np.asarray(inputs["Wcq"], np.float32)
    Wckv = np.asarray(inputs["Wckv"], np.float32)
    Wp = np.asarray(inputs["Wproj"], np.float32)

    per_batch = []
    for b in range(B):
        xb = x[b]
        glog = (xb @ np.asarray(inputs["Wgate"], np.float32)).mean(0)
        g = np.exp(glog - glog.max())
        g = (g / g.sum()).astype(np.float32)
        scores = (xb @ np.asarray(inputs["W_imp"], np.float32))[:, 0]
        idx = np.sort(np.argpartition(-scores, KEEP - 1)[:KEEP])
        per_batch.append((xb, xb[idx], g))

    in_maps = []
    for core in range(N_CORES):
        b, hg2 = divmod(core, N_CORES // B)
        xb, sel, g = per_batch[b]

        blob = np.zeros((128, NB), np.float32)

        def put(col, w):  # w [C, X] -> 2 cin chunks side by side
            Xw = w.shape[1]
            for cc in range(2):
                blob[:, col + cc * Xw : col + (cc + 1) * Xw] = \
                    w[cc * 128 : (cc + 1) * 128]

        put(XT, xb.T)
        put(SEL, sel.T)
        b96_parts, b32_parts = [], [c32, s32]
        for hq in range(QUADS):
            gq = hg2 * QUADS + hq  # global head-quad
            hsl = slice(gq * HPC, gq * HPC + HPC)
            bo = hq * QW
            put(bo + WSKN, Wsk_r[:, hsl, :NOPE].reshape(C, -1))
            put(bo + WSKR, Wsk_r[:, hsl, NOPE:].reshape(C, -1))
            put(bo + WSV, (Wsv_r[:, hsl] * g[1]).reshape(C, -1))
            put(bo + WWKN, Wwk_r[:, hsl, :NOPE].reshape(C, -1))
            put(bo + WWKR, Wwk_r[:, hsl, NOPE:].reshape(C, -1))
            put(bo + WWV, (Wwv_r[:, hsl] * g[2]).reshape(C, -1))
            b96_parts += [Wqn_r[:, hsl].reshape(96, -1),
                          Wqr_r[:, hsl].reshape(96, -1)]
            b32_parts += [Wkn_r[:, hsl].reshape(32, -1),
                          (Wv_r[:, hsl] * g[0]).reshape(32, -1)]
        put(WCQ, Wcq)
        put(WCKV, Wckv)
        put(WKR, Wkr)

        b96 = np.concatenate(b96_parts, 1)
        b32 = np.concatenate(b32_parts, 1)

        in_maps.append({
            "blob": blob.astype(BF),
            "b96": np.ascontiguousarray(b96).astype(BF),
            "b32": np.ascontiguousarray(b32).astype(BF),
        })
    return in_maps, Wp


def _run(inputs, trace=False):
    from concourse.bass_utils import run_bass_kernel_spmd

    nc = _get_nc()
    in_maps, Wp = _prep_in_maps(inputs)
    res = run_bass_kernel_spmd(nc, in_maps, list(range(N_CORES)), trace=trace)
    out = np.zeros((B, T, C), np.float32)
    for core in range(N_CORES):
        b, hg2 = divmod(core, N_CORES // B)
        accs = np.asarray(res.results[core]["outT"], np.float32)  # [2,128,T]
        for hq in range(QUADS):
            gq = hg2 * QUADS + hq
            out[b] += accs[hq].T @ Wp[gq * 128 : gq * 128 + 128]
    return out, res


def kernel(**inputs):
    out, _ = _run(inputs, trace=False)
    return out
